# revision 9
# baseline (speedup 1.0000x reference)
"""Trainium2 Bass kernel for 8-head causal MultiHeadAttention.

Problem (hardcoded): B=8, S=1024, d_model=512, H=8, d_k=128, d_v=256,
causal sequence mask, all-ones padding mask, fp32.

Strategy:
  - Batch-parallel across the 8 NeuronCores (1 batch element per core).
  - All matmuls in float32r (TF32-like fp32 @ 4x fp32 rate; ~13 mantissa
    bits) with every matmul free dim >= 256 for the full 1 cycle/row rate.
  - Scores are computed TRANSPOSED (S^T[t, q]) so the P@V contraction needs
    no transposes of the attention matrix; softmax denominators via a
    ones-vector matmul; normalization applied to O^T after the PV matmul;
    reciprocal broadcast across partitions with a rank-1 PE matmul.
  - Causality handled structurally (lower-triangular t-tiles only) plus
    0/1 mask multiplies on diagonal-band blocks.
  - Host side: transposes Q/K/V per batch element (so the kernel DMAs are
    contiguous), folds bv through softmax (rows sum to 1) and bo into a
    single host-side bias add, and transposes the per-core out^T back.
"""

import numpy as np

import concourse.bacc as bacc
import concourse.mybir as mybir
from concourse import tile
from concourse.bass_utils import run_bass_kernel_spmd
from concourse.tile_rust import add_dep_helper

B, S, D, H, DK, DV = 8, 1024, 512, 8, 128, 256
F32 = mybir.dt.float32
F32R = mybir.dt.float32r
ACT = mybir.ActivationFunctionType
SCALE = float(np.float32(1.0) / np.sqrt(np.float32(DK)).astype(np.float32))

_CACHE = {}


def build():
    nc = bacc.Bacc(trn_type="TRN2", target_bir_lowering=False, debug=False)

    qT_d = nc.dram_tensor("qT", [D, S], F32R, kind="ExternalInput").ap()
    kT_d = nc.dram_tensor("kT", [D, S], F32R, kind="ExternalInput").ap()
    vT_d = nc.dram_tensor("vT", [D, S], F32R, kind="ExternalInput").ap()
    wq_d = nc.dram_tensor("wq", [H, D, DK], F32R, kind="ExternalInput").ap()
    wk_d = nc.dram_tensor("wk", [H, D, DK], F32R, kind="ExternalInput").ap()
    wv_d = nc.dram_tensor("wv", [H, D, DV], F32R, kind="ExternalInput").ap()
    wo_d = nc.dram_tensor("wo", [H * DV, D], F32R, kind="ExternalInput").ap()
    bq_d = nc.dram_tensor("bq", [H, DK], F32, kind="ExternalInput").ap()
    bk_d = nc.dram_tensor("bk", [H, DK], F32, kind="ExternalInput").ap()
    mask_d = nc.dram_tensor("maskblk", [4, 128, 512], F32, kind="ExternalInput").ap()
    onescol_d = nc.dram_tensor("onescol", [128, 1], F32R, kind="ExternalInput").ap()
    onesrow_d = nc.dram_tensor("onesrow", [1, 128], F32R, kind="ExternalInput").ap()
    outT_d = nc.dram_tensor("outT", [D, S], F32, kind="ExternalOutput").ap()

    with tile.TileContext(nc) as tc:
        with (
            tc.tile_pool(name="const", bufs=1) as const,
            tc.tile_pool(name="oTp", bufs=1) as oTp,
            tc.tile_pool(name="whead", bufs=2) as whead,
            tc.tile_pool(name="proj", bufs=1) as proj,
            tc.tile_pool(name="ptp", bufs=4) as ptp,
            tc.tile_pool(name="wop", bufs=4) as wop,
            tc.tile_pool(name="outst", bufs=2) as outst,
            tc.tile_pool(name="recipp", bufs=2) as recipp,
        ):
            attn_psum = tc.tile_pool(name="ps_a", bufs=2, space="PSUM")
            ps_a = attn_psum.__enter__()
            _ps_s_cm = tc.tile_pool(name="ps_s", bufs=2, space="PSUM")
            ps_s = _ps_s_cm.__enter__()
            _ps_acc_cm = tc.tile_pool(name="ps_acc", bufs=3, space="PSUM")
            ps_acc = _ps_acc_cm.__enter__()

            # ---- resident inputs ----
            qTs, kTs, vTs = [], [], []
            for name, dram, lst in (("q", qT_d, qTs), ("k", kT_d, kTs), ("v", vT_d, vTs)):
                for k in range(4):
                    t = const.tile([128, S], F32R, tag=f"{name}T{k}")
                    nc.sync.dma_start(t[:], dram[128 * k : 128 * k + 128, :])
                    lst.append(t)
            mask_s = const.tile([128, 4 * 512], F32, tag="maskblk")
            nc.sync.dma_start(
                mask_s[:].rearrange("p (r n) -> p r n", r=4),
                mask_d.rearrange("r p n -> p r n"),
            )
            onescol = const.tile([128, 1], F32R, tag="onescol")
            nc.sync.dma_start(onescol[:], onescol_d[:])
            onesrow = const.tile([1, 128], F32R, tag="onesrow")
            nc.sync.dma_start(onesrow[:], onesrow_d[:])

            oT = [oTp.tile([128, S], F32R, tag=f"oT{i}", name=f"oT{i}") for i in range(16)]

            # ---- per-head projections + attention ----
            for h in range(H):
                wq_s = whead.tile([128, 4 * DK], F32R, tag="wq")
                nc.sync.dma_start(
                    wq_s[:].rearrange("p (k m) -> p k m", k=4),
                    wq_d[h].rearrange("(k p) m -> p k m", p=128),
                )
                wk_s = whead.tile([128, 4 * DK], F32R, tag="wk")
                nc.sync.dma_start(
                    wk_s[:].rearrange("p (k m) -> p k m", k=4),
                    wk_d[h].rearrange("(k p) m -> p k m", p=128),
                )
                wv_s = whead.tile([128, 4 * DV], F32R, tag="wv")
                nc.sync.dma_start(
                    wv_s[:].rearrange("p (k m) -> p k m", k=4),
                    wv_d[h].rearrange("(k p) m -> p k m", p=128),
                )
                bq_s = whead.tile([128, 1], F32, tag="bq")
                nc.sync.dma_start(bq_s[:], bq_d[h : h + 1, :].rearrange("o p -> p o"))
                bk_s = whead.tile([128, 1], F32, tag="bk")
                nc.sync.dma_start(bk_s[:], bk_d[h : h + 1, :].rearrange("o p -> p o"))

                # Qp^T / Kp^T: [dk=128, S]
                qpT = proj.tile([128, S], F32R, tag="qpT")
                kpT = proj.tile([128, S], F32R, tag="kpT")
                for dst, w_s, src, b_s in ((qpT, wq_s, qTs, bq_s), (kpT, wk_s, kTs, bk_s)):
                    for c in range(2):
                        p = ps_a.tile([128, 512], F32, tag="pa")
                        for k in range(4):
                            nc.tensor.matmul(
                                p[:],
                                w_s[:, 128 * k : 128 * k + 128],
                                src[k][:, 512 * c : 512 * c + 512],
                                start=(k == 0),
                                stop=(k == 3),
                            )
                        nc.vector.tensor_scalar_add(
                            dst[:, 512 * c : 512 * c + 512], p[:], b_s[:]
                        )

                # Vp: [t, v] stored as [128, 8*256]
                vp = proj.tile([128, 8 * DV], F32R, tag="vp")
                for i in range(8):
                    p = ps_a.tile([128, DV], F32, tag="pa")
                    for k in range(4):
                        nc.tensor.matmul(
                            p[:],
                            vTs[k][:, 128 * i : 128 * i + 128],
                            wv_s[:, DV * k : DV * k + DV],
                            start=(k == 0),
                            stop=(k == 3),
                        )
                    nc.scalar.activation(vp[:, DV * i : DV * i + DV], p[:], ACT.Copy)

                # attention per 512-wide q-chunk
                for j in range(2):
                    n_t = 4 * (j + 1)
                    po = [
                        ps_acc.tile([128, 512], F32, tag="acc", name=f"po{vh}")
                        for vh in range(2)
                    ]
                    pr = ps_acc.tile([1, 512], F32, tag="acc")
                    for i in range(n_t):
                        psc = ps_s.tile([128, 512], F32, tag="ps")
                        nc.tensor.matmul(
                            psc[:],
                            kpT[:, 128 * i : 128 * i + 128],
                            qpT[:, 512 * j : 512 * j + 512],
                            start=True,
                            stop=True,
                        )
                        pt = ptp.tile([128, 512], F32R, tag="pt")
                        nc.scalar.activation(pt[:], psc[:], ACT.Exp, scale=SCALE)
                        if i >= 4 * j:
                            r = i - 4 * j
                            nc.vector.tensor_mul(
                                pt[:], pt[:], mask_s[:, 512 * r : 512 * r + 512]
                            )
                        for vh in range(2):
                            nc.tensor.matmul(
                                po[vh][:],
                                vp[:, DV * i + 128 * vh : DV * i + 128 * vh + 128],
                                pt[:],
                                start=(i == 0),
                                stop=(i == n_t - 1),
                            )
                        nc.tensor.matmul(
                            pr[:],
                            onescol[:],
                            pt[:],
                            start=(i == 0),
                            stop=(i == n_t - 1),
                        )
                    recip = recipp.tile([1, 512], F32R, tag="recip")
                    with nc.allow_low_precision(reason="f32r keeps fp32 class"):
                        nc.vector.reciprocal(recip[:], pr[:])
                    pb = ps_s.tile([128, 512], F32, tag="ps")
                    nc.tensor.matmul(pb[:], onesrow[:], recip[:], start=True, stop=True)
                    pbs = recipp.tile([128, 512], F32, tag="pbs")
                    nc.scalar.activation(pbs[:], pb[:], ACT.Copy)
                    for vh in range(2):
                        mm = nc.vector.tensor_mul(
                            oT[2 * h + vh][:, 512 * j : 512 * j + 512], po[vh][:], pbs[:]
                        )
                        last_attn = mm

            _ps_acc_cm.__exit__(None, None, None)
            _ps_s_cm.__exit__(None, None, None)
            attn_psum.__exit__(None, None, None)
            _ps_out_cm = tc.tile_pool(name="ps_out", bufs=8, space="PSUM")
            ps_out = _ps_out_cm.__enter__()

            # ---- output projection: outT[m, s] ----
            # kk outer so each wo tile is consumed in one burst (4 wop slots
            # suffice); 8 psum accumulators live, gated behind the end of
            # attention so PSUM banks never overcommit.
            po8 = [
                ps_out.tile([128, 512], F32, tag="pout", name=f"pout{g}")
                for g in range(8)
            ]
            for kk in range(16):
                w = wop.tile([128, D], F32R, tag="wo", name=f"wo{kk}")
                nc.sync.dma_start(w[:], wo_d[128 * kk : 128 * kk + 128, :])
                for g in range(8):
                    m, c = divmod(g, 2)
                    mm = nc.tensor.matmul(
                        po8[g][:],
                        w[:, 128 * m : 128 * m + 128],
                        oT[kk][:, 512 * c : 512 * c + 512],
                        start=(kk == 0),
                        stop=(kk == 15),
                    )
                    if kk == 0:
                        add_dep_helper(
                            mm.ins,
                            last_attn.ins,
                            sync=False,
                            reason="out-proj psum after attention psum freed",
                        )
            for g in range(8):
                m, c = divmod(g, 2)
                st = outst.tile([128, 512], F32, tag="outst")
                nc.scalar.activation(st[:], po8[g][:], ACT.Copy)
                nc.sync.dma_start(
                    outT_d[128 * m : 128 * m + 128, 512 * c : 512 * c + 512], st[:]
                )
            _ps_out_cm.__exit__(None, None, None)

    nc.compile()
    return nc


def _prep(Q, K, V, padding_mask, sequence_mask, Wq, bq, Wk, bk, Wv, bv, Wo, bo):
    assert padding_mask.min() == 1, "kernel assumes all-ones padding mask"
    seq = np.asarray(sequence_mask)
    maskblk = np.zeros((4, 128, 512), np.float32)
    for r in range(4):
        blk0 = seq[0:512, 128 * r : 128 * r + 128].T.astype(np.float32)
        blk1 = seq[512:1024, 128 * (4 + r) : 128 * (4 + r) + 128].T.astype(np.float32)
        assert np.array_equal(blk0, blk1), "kernel assumes causal sequence mask"
        maskblk[r] = blk0
    c = np.ascontiguousarray
    shared = {
        "wq": c(Wq.astype(np.float32)),
        "wk": c(Wk.astype(np.float32)),
        "wv": c(Wv.astype(np.float32)),
        "wo": c(Wo.astype(np.float32)),
        "bq": c(bq.astype(np.float32)),
        "bk": c(bk.astype(np.float32)),
        "maskblk": maskblk,
        "onescol": np.ones((128, 1), np.float32),
        "onesrow": np.ones((1, 128), np.float32),
    }
    in_maps = []
    for b in range(B):
        m = dict(shared)
        m["qT"] = c(np.asarray(Q[b]).T.astype(np.float32))
        m["kT"] = c(np.asarray(K[b]).T.astype(np.float32))
        m["vT"] = c(np.asarray(V[b]).T.astype(np.float32))
        in_maps.append(m)
    bo_eff = (
        np.asarray(bo, np.float32)
        + np.asarray(bv, np.float32).reshape(H * DV) @ np.asarray(Wo, np.float32)
    ).astype(np.float32)
    return in_maps, bo_eff


def kernel(Q, K, V, padding_mask, sequence_mask, Wq, bq, Wk, bk, Wv, bv, Wo, bo):
    if "nc" not in _CACHE:
        _CACHE["nc"] = build()
    nc = _CACHE["nc"]
    in_maps, bo_eff = _prep(
        Q, K, V, padding_mask, sequence_mask, Wq, bq, Wk, bk, Wv, bv, Wo, bo
    )
    res = run_bass_kernel_spmd(nc, in_maps, core_ids=list(range(B)))
    out = np.empty((B, S, D), np.float32)
    for b in range(B):
        out[b] = res.results[b]["outT"].T + bo_eff
    return out


# revision 39
# speedup vs baseline: 62.9416x; 62.9416x over previous
"""Trainium2 Bass kernel for 8-head causal MultiHeadAttention.

Problem (hardcoded): B=8, S=1024, d_model=512, H=8, d_k=128, d_v=256,
causal sequence mask, all-ones padding mask, fp32.

Strategy:
  - Batch-parallel across the 8 NeuronCores (1 batch element per core).
  - All matmuls in float32r (TF32-like fp32 @ 4x fp32 rate; ~13 mantissa
    bits) with every matmul free dim >= 256 for the full 1 cycle/row rate.
  - Scores are computed TRANSPOSED (S^T[t, q]) so the P@V contraction needs
    no transposes of the attention matrix; softmax denominators via a
    ones-vector matmul accumulated alongside PV; normalization applied to
    O^T after the PV matmul; the reciprocal is broadcast across partitions
    on the (otherwise idle) GPSIMD engine.
  - Causality handled structurally: only lower-triangular t-tiles are
    computed, diagonal-band blocks are trapezoid-narrowed to the live
    column window (>=256 wide to keep f32r at full rate), and a single
    resident [zeros|tril|ones] mask strip provides every diagonal mask via
    windowed in-place multiplies on just the nontrivial columns.
  - Host side: transposes Q/K/V per batch element (so the kernel DMAs are
    contiguous), folds bv through softmax (rows sum to 1) and bo into a
    single host-side bias add, and transposes the per-core out^T back.
"""

import numpy as np

import concourse.bacc as bacc
import concourse.mybir as mybir
from concourse import tile
from concourse.bass_utils import run_bass_kernel_spmd
from concourse.tile_rust import add_dep_helper

B, S, D, H, DK, DV = 8, 1024, 512, 8, 128, 256
F32 = mybir.dt.float32
F32R = mybir.dt.float32r
ACT = mybir.ActivationFunctionType
SCALE = float(np.float32(1.0) / np.sqrt(np.float32(DK)).astype(np.float32))

_CACHE = {}


def build():
    nc = bacc.Bacc(trn_type="TRN2", target_bir_lowering=False, debug=False)

    qT_d = nc.dram_tensor("qT", [D, S], F32R, kind="ExternalInput").ap()
    kT_d = nc.dram_tensor("kT", [D, S], F32R, kind="ExternalInput").ap()
    vT_d = nc.dram_tensor("vT", [D, S], F32R, kind="ExternalInput").ap()
    wq_d = nc.dram_tensor("wq", [H, D, DK], F32R, kind="ExternalInput").ap()
    wk_d = nc.dram_tensor("wk", [H, D, DK], F32R, kind="ExternalInput").ap()
    wv_d = nc.dram_tensor("wv", [H, D, DV], F32R, kind="ExternalInput").ap()
    wo_d = nc.dram_tensor("wo", [H * DV, D], F32R, kind="ExternalInput").ap()
    bq_d = nc.dram_tensor("bq", [H, DK], F32, kind="ExternalInput").ap()
    bk_d = nc.dram_tensor("bk", [H, DK], F32, kind="ExternalInput").ap()
    mask_d = nc.dram_tensor("maskstrip", [128, 896], F32, kind="ExternalInput").ap()
    onescol_d = nc.dram_tensor("onescol", [128, 1], F32R, kind="ExternalInput").ap()
    outT_d = nc.dram_tensor("outT", [D, S], F32, kind="ExternalOutput").ap()

    with tile.TileContext(nc) as tc:
        with (
            tc.tile_pool(name="const", bufs=1) as const,
            tc.tile_pool(name="oTp", bufs=1) as oTp,
            tc.tile_pool(name="whead", bufs=2) as whead,
            tc.tile_pool(name="proj", bufs=2) as proj,
            tc.tile_pool(name="ptp", bufs=6) as ptp,
            tc.tile_pool(name="wop", bufs=8) as wop,
            tc.tile_pool(name="outst", bufs=2) as outst,
            tc.tile_pool(name="recipp", bufs=2) as recipp,
        ):
            attn_psum = tc.tile_pool(name="ps_a", bufs=2, space="PSUM")
            ps_a = attn_psum.__enter__()
            _ps_s_cm = tc.tile_pool(name="ps_s", bufs=3, space="PSUM")
            ps_s = _ps_s_cm.__enter__()
            _ps_acc_cm = tc.tile_pool(name="ps_acc", bufs=3, space="PSUM")
            ps_acc = _ps_acc_cm.__enter__()

            # ---- resident inputs ----
            def load_head_weights(h):
                wq_s = whead.tile([128, 4 * DK], F32R, tag="wq", name=f"wq{h}")
                nc.sync.dma_start(
                    wq_s[:].rearrange("p (k m) -> p k m", k=4),
                    wq_d[h].rearrange("(k p) m -> p k m", p=128),
                )
                wk_s = whead.tile([128, 4 * DK], F32R, tag="wk", name=f"wk{h}")
                nc.sync.dma_start(
                    wk_s[:].rearrange("p (k m) -> p k m", k=4),
                    wk_d[h].rearrange("(k p) m -> p k m", p=128),
                )
                wv_s = whead.tile([128, 4 * DV], F32R, tag="wv", name=f"wv{h}")
                nc.sync.dma_start(
                    wv_s[:].rearrange("p (k m) -> p k m", k=4),
                    wv_d[h].rearrange("(k p) m -> p k m", p=128),
                )
                bq_s = whead.tile([128, 1], F32, tag="bq", name=f"bq{h}")
                nc.sync.dma_start(bq_s[:], bq_d[h : h + 1, :].rearrange("o p -> p o"))
                bk_s = whead.tile([128, 1], F32, tag="bk", name=f"bk{h}")
                nc.sync.dma_start(bk_s[:], bk_d[h : h + 1, :].rearrange("o p -> p o"))
                return wq_s, wk_s, wv_s, bq_s, bk_s

            qTs, kTs, vTs = [], [], []
            for name, dram, lst in (("q", qT_d, qTs), ("k", kT_d, kTs), ("v", vT_d, vTs)):
                for k in range(4):
                    t = const.tile([128, S], F32R, tag=f"{name}T{k}", name=f"{name}T{k}")
                    lst.append(t)
            # startup-ordered loads: each projection's weight right before
            # the input tensor it contracts with
            wq_s0 = whead.tile([128, 4 * DK], F32R, tag="wq", name="wq0")
            nc.sync.dma_start(
                wq_s0[:].rearrange("p (k m) -> p k m", k=4),
                wq_d[0].rearrange("(k p) m -> p k m", p=128),
            )
            for k in range(4):
                nc.sync.dma_start(qTs[k][:], qT_d[128 * k : 128 * k + 128, :])
            wk_s0 = whead.tile([128, 4 * DK], F32R, tag="wk", name="wk0")
            nc.sync.dma_start(
                wk_s0[:].rearrange("p (k m) -> p k m", k=4),
                wk_d[0].rearrange("(k p) m -> p k m", p=128),
            )
            for k in range(4):
                nc.sync.dma_start(kTs[k][:], kT_d[128 * k : 128 * k + 128, :])
            wv_s0 = whead.tile([128, 4 * DV], F32R, tag="wv", name="wv0")
            nc.sync.dma_start(
                wv_s0[:].rearrange("p (k m) -> p k m", k=4),
                wv_d[0].rearrange("(k p) m -> p k m", p=128),
            )
            bq_s0 = whead.tile([128, 1], F32, tag="bq", name="bq0")
            nc.sync.dma_start(bq_s0[:], bq_d[0:1, :].rearrange("o p -> p o"))
            bk_s0 = whead.tile([128, 1], F32, tag="bk", name="bk0")
            nc.sync.dma_start(bk_s0[:], bk_d[0:1, :].rearrange("o p -> p o"))
            head1_weights = load_head_weights(1)
            for k in range(4):
                nc.sync.dma_start(vTs[k][:], vT_d[128 * k : 128 * k + 128, :])
            head0_weights = (wq_s0, wk_s0, wv_s0, bq_s0, bk_s0)
            mask_s = const.tile([128, 896], F32, tag="maskstrip")
            nc.sync.dma_start(mask_s[:], mask_d[:])
            onescol = const.tile([128, 1], F32R, tag="onescol")
            nc.sync.dma_start(onescol[:], onescol_d[:])

            oT = [oTp.tile([128, S], F32R, tag=f"oT{i}", name=f"oT{i}") for i in range(16)]

            # ---- per-head projections + attention (software-pipelined:
            # head h+1's Q/K projections are emitted before head h's V
            # projection so the PE never queues behind vT-gated work) ----
            def proj_qk(h, weights):
                wq_s, wk_s, _, bq_s, bk_s = weights
                qpT = proj.tile([128, S], F32R, tag="qpT", name=f"qpT{h}")
                kpT = proj.tile([128, S], F32R, tag="kpT", name=f"kpT{h}")
                for dst, w_s, src, b_s in ((qpT, wq_s, qTs, bq_s), (kpT, wk_s, kTs, bk_s)):
                    for c in range(2):
                        p = ps_a.tile([128, 512], F32, tag="pa")
                        for k in range(4):
                            nc.tensor.matmul(
                                p[:],
                                w_s[:, 128 * k : 128 * k + 128],
                                src[k][:, 512 * c : 512 * c + 512],
                                start=(k == 0),
                                stop=(k == 3),
                            )
                        if c == 0:
                            nc.scalar.activation(
                                dst[:, 512 * c : 512 * c + 512], p[:], ACT.Identity,
                                bias=b_s[:],
                            )
                        else:
                            nc.vector.tensor_scalar_add(
                                dst[:, 512 * c : 512 * c + 512], p[:], b_s[:]
                            )
                return qpT, kpT

            def proj_v(h, weights):
                wv_s = weights[2]
                vp = proj.tile([128, 8 * DV], F32R, tag="vp", name=f"vp{h}")
                for i in range(8):
                    p = ps_a.tile([128, DV], F32, tag="pa")
                    for k in range(4):
                        nc.tensor.matmul(
                            p[:],
                            vTs[k][:, 128 * i : 128 * i + 128],
                            wv_s[:, DV * k : DV * k + DV],
                            start=(k == 0),
                            stop=(k == 3),
                        )
                    if i % 2 == 0:
                        nc.scalar.activation(
                            vp[:, DV * i : DV * i + DV], p[:], ACT.Copy
                        )
                    else:
                        nc.vector.tensor_copy(vp[:, DV * i : DV * i + DV], p[:])
                return vp

            def attn(h, qpT, kpT, vp):
                last = None
                # attention per 512-wide q-chunk
                for j in range(2):
                    n_t = 4 * (j + 1)
                    qlo = 512 * j
                    po = [
                        ps_acc.tile([128, 512], F32, tag="acc", name=f"po{vh}")
                        for vh in range(2)
                    ]
                    pr = ps_acc.tile([1, 512], F32, tag="acc")
                    for i in range(n_t):
                        # live column window: causality kills q < 128*r in
                        # this t-tile; round the window down to >=256 wide so
                        # f32r stays at full rate
                        r = i - 4 * j
                        wlo = 0 if r < 1 else min(128 * r, 256)
                        nw = 512 - wlo
                        psc = ps_s.tile([128, nw], F32, tag="ps", name=f"psc{i}")
                        nc.tensor.matmul(
                            psc[:],
                            kpT[:, 128 * i : 128 * i + 128],
                            qpT[:, qlo + wlo : qlo + 512],
                            start=True,
                            stop=True,
                        )
                        pt = ptp.tile([128, nw], F32R, tag="pt", name=f"pt{i}")
                        nc.scalar.activation(pt[:], psc[:], ACT.Exp, scale=SCALE)
                        if 0 <= r <= 2:
                            lo = 128 * r - wlo
                            nc.vector.tensor_mul(
                                pt[:, lo : lo + 128],
                                pt[:, lo : lo + 128],
                                mask_s[:, 384:512],
                            )
                        elif r == 3:
                            nc.vector.tensor_mul(
                                pt[:, 0:256],
                                pt[:, 0:256],
                                mask_s[:, 256:512],
                            )
                        for vh in range(2):
                            nc.tensor.matmul(
                                po[vh][:, wlo:512],
                                vp[:, DV * i + 128 * vh : DV * i + 128 * vh + 128],
                                pt[:],
                                start=(i == 0),
                                stop=(i == n_t - 1),
                                skip_group_check=True,
                            )
                        nc.tensor.matmul(
                            pr[:, wlo:512],
                            onescol[:],
                            pt[:],
                            start=(i == 0),
                            stop=(i == n_t - 1),
                            skip_group_check=True,
                        )
                    recip = recipp.tile([1, 512], F32, tag="recip")
                    nc.vector.reciprocal(recip[:], pr[:])
                    pbs = recipp.tile([128, 512], F32, tag="pbs")
                    nc.gpsimd.partition_broadcast(pbs[:], recip[:], 128)
                    for vh in range(2):
                        mm = nc.vector.tensor_mul(
                            oT[2 * h + vh][:, qlo : qlo + 512], po[vh][:], pbs[:]
                        )
                        last = mm
                return last

            weights = {0: head0_weights, 1: head1_weights}
            for h in range(H):
                if h not in weights:
                    weights[h] = load_head_weights(h)
                qpT_h, kpT_h = proj_qk(h, weights[h])
                vp_h = proj_v(h, weights[h])
                last_attn = attn(h, qpT_h, kpT_h, vp_h)

            # ---- output projection: outT[m, s] ----
            # kk outer so each wo tile is consumed in one burst (4 wop slots
            # suffice); 8 psum accumulators live, gated behind the end of
            # attention so PSUM banks never overcommit.
            _pools8 = [ps_a, ps_a, ps_s, ps_s, ps_s, ps_acc, ps_acc, ps_acc]
            _tags8 = ["pa", "pa", "ps", "ps", "ps", "acc", "acc", "acc"]
            po8 = [
                _pools8[g].tile([128, 512], F32, tag=_tags8[g], name=f"pout{g}")
                for g in range(8)
            ]
            # phase A: kk-outer over first half of the contraction
            wo_tiles = {}
            for kk in range(8):
                w = wop.tile([128, D], F32R, tag="wo", name=f"wo{kk}")
                nc.sync.dma_start(w[:], wo_d[128 * kk : 128 * kk + 128, :])
                for g in range(8):
                    m, c = divmod(g, 2)
                    mm = nc.tensor.matmul(
                        po8[g][:],
                        w[:, 128 * m : 128 * m + 128],
                        oT[kk][:, 512 * c : 512 * c + 512],
                        start=(kk == 0),
                        stop=False,
                    )
                    if kk == 0:
                        add_dep_helper(
                            mm.ins,
                            last_attn.ins,
                            sync=False,
                            reason="out-proj psum after attention psum freed",
                        )
            # phase B: group-major so early groups finish, evict and DMA out
            # while later groups still accumulate
            for kk in range(8, 16):
                w = wop.tile([128, D], F32R, tag="wo", name=f"wo{kk}")
                nc.sync.dma_start(w[:], wo_d[128 * kk : 128 * kk + 128, :])
                wo_tiles[kk] = w
            for g in range(8):
                m, c = divmod(g, 2)
                for kk in range(8, 16):
                    nc.tensor.matmul(
                        po8[g][:],
                        wo_tiles[kk][:, 128 * m : 128 * m + 128],
                        oT[kk][:, 512 * c : 512 * c + 512],
                        start=False,
                        stop=(kk == 15),
                    )
                st = outst.tile([128, 512], F32, tag="outst")
                nc.scalar.activation(st[:], po8[g][:], ACT.Copy)
                nc.sync.dma_start(
                    outT_d[128 * m : 128 * m + 128, 512 * c : 512 * c + 512], st[:]
                )
            _ps_acc_cm.__exit__(None, None, None)
            _ps_s_cm.__exit__(None, None, None)
            attn_psum.__exit__(None, None, None)

    nc.compile()
    return nc


def _prep(Q, K, V, padding_mask, sequence_mask, Wq, bq, Wk, bk, Wv, bv, Wo, bo):
    assert padding_mask.min() == 1, "kernel assumes all-ones padding mask"
    seq = np.asarray(sequence_mask)
    tril = seq[0:128, 0:128].T.astype(np.float32)
    maskstrip = np.concatenate(
        [np.zeros((128, 384), np.float32), tril, np.ones((128, 384), np.float32)],
        axis=1,
    )
    for j in range(2):
        for i in range(4 * j, 4 * j + 4):
            r = i - 4 * j
            blk = seq[
                512 * j : 512 * j + 512, 128 * i : 128 * i + 128
            ].T.astype(np.float32)
            assert np.array_equal(
                blk, maskstrip[:, 384 - 128 * r : 896 - 128 * r]
            ), "kernel assumes causal sequence mask"
        for i in range(4 * j):
            assert seq[512 * j : 512 * j + 512, 128 * i : 128 * i + 128].min() == 1
    c = np.ascontiguousarray
    shared = {
        "wq": c(Wq.astype(np.float32)),
        "wk": c(Wk.astype(np.float32)),
        "wv": c(Wv.astype(np.float32)),
        "wo": c(Wo.astype(np.float32)),
        "bq": c(bq.astype(np.float32)),
        "bk": c(bk.astype(np.float32)),
        "maskstrip": maskstrip,
        "onescol": np.ones((128, 1), np.float32),
    }
    in_maps = []
    for b in range(B):
        m = dict(shared)
        m["qT"] = c(np.asarray(Q[b]).T.astype(np.float32))
        m["kT"] = c(np.asarray(K[b]).T.astype(np.float32))
        m["vT"] = c(np.asarray(V[b]).T.astype(np.float32))
        in_maps.append(m)
    bo_eff = (
        np.asarray(bo, np.float32)
        + np.asarray(bv, np.float32).reshape(H * DV) @ np.asarray(Wo, np.float32)
    ).astype(np.float32)
    return in_maps, bo_eff


def kernel(Q, K, V, padding_mask, sequence_mask, Wq, bq, Wk, bk, Wv, bv, Wo, bo):
    if "nc" not in _CACHE:
        _CACHE["nc"] = build()
    nc = _CACHE["nc"]
    in_maps, bo_eff = _prep(
        Q, K, V, padding_mask, sequence_mask, Wq, bq, Wk, bk, Wv, bv, Wo, bo
    )
    res = run_bass_kernel_spmd(nc, in_maps, core_ids=list(range(B)))
    out = np.empty((B, S, D), np.float32)
    for b in range(B):
        out[b] = res.results[b]["outT"].T + bo_eff
    return out


# revision 44
# speedup vs baseline: 63.1918x; 1.0040x over previous
"""Trainium2 Bass kernel for 8-head causal MultiHeadAttention.

Problem (hardcoded): B=8, S=1024, d_model=512, H=8, d_k=128, d_v=256,
causal sequence mask, all-ones padding mask, fp32.

Strategy:
  - Batch-parallel across the 8 NeuronCores (1 batch element per core).
  - All matmuls in float32r (TF32-like fp32 @ 4x fp32 rate; ~13 mantissa
    bits) with every matmul free dim >= 256 for the full 1 cycle/row rate.
  - Scores are computed TRANSPOSED (S^T[t, q]) so the P@V contraction needs
    no transposes of the attention matrix; softmax denominators via a
    ones-vector matmul; normalization applied to O^T after the PV matmul;
    reciprocal broadcast across partitions with a rank-1 PE matmul.
  - Causality handled structurally (lower-triangular t-tiles only) plus
    0/1 mask multiplies on diagonal-band blocks.
  - Host side: transposes Q/K/V per batch element (so the kernel DMAs are
    contiguous), folds bv through softmax (rows sum to 1) and bo into a
    single host-side bias add, and transposes the per-core out^T back.
"""

import numpy as np

import concourse.bacc as bacc
import concourse.mybir as mybir
from concourse import tile
from concourse.bass_utils import run_bass_kernel_spmd
from concourse.tile_rust import add_dep_helper

B, S, D, H, DK, DV = 8, 1024, 512, 8, 128, 256
F32 = mybir.dt.float32
F32R = mybir.dt.float32r
ACT = mybir.ActivationFunctionType
SCALE = float(np.float32(1.0) / np.sqrt(np.float32(DK)).astype(np.float32))

_CACHE = {}


def build():
    nc = bacc.Bacc(trn_type="TRN2", target_bir_lowering=False, debug=False)

    qT_d = nc.dram_tensor("qT", [D, S], F32R, kind="ExternalInput").ap()
    kT_d = nc.dram_tensor("kT", [D, S], F32R, kind="ExternalInput").ap()
    vT_d = nc.dram_tensor("vT", [D, S], F32R, kind="ExternalInput").ap()
    wq_d = nc.dram_tensor("wq", [H, D, DK], F32R, kind="ExternalInput").ap()
    wk_d = nc.dram_tensor("wk", [H, D, DK], F32R, kind="ExternalInput").ap()
    wv_d = nc.dram_tensor("wv", [H, D, DV], F32R, kind="ExternalInput").ap()
    wo_d = nc.dram_tensor("wo", [H * DV, D], F32R, kind="ExternalInput").ap()
    bq_d = nc.dram_tensor("bq", [H, DK], F32, kind="ExternalInput").ap()
    bk_d = nc.dram_tensor("bk", [H, DK], F32, kind="ExternalInput").ap()
    mask_d = nc.dram_tensor("maskstrip", [128, 896], F32, kind="ExternalInput").ap()
    onescol_d = nc.dram_tensor("onescol", [128, 1], F32R, kind="ExternalInput").ap()
    outT_d = nc.dram_tensor("outT", [D, S], F32, kind="ExternalOutput").ap()

    with tile.TileContext(nc) as tc:
        with (
            tc.tile_pool(name="const", bufs=1) as const,
            tc.tile_pool(name="oTp", bufs=1) as oTp,
            tc.tile_pool(name="whead", bufs=2) as whead,
            tc.tile_pool(name="proj", bufs=2) as proj,
            tc.tile_pool(name="ptp", bufs=6) as ptp,
            tc.tile_pool(name="wop", bufs=8) as wop,
            tc.tile_pool(name="outst", bufs=2) as outst,
            tc.tile_pool(name="recipp", bufs=2) as recipp,
        ):
            attn_psum = tc.tile_pool(name="ps_a", bufs=2, space="PSUM")
            ps_a = attn_psum.__enter__()
            _ps_s_cm = tc.tile_pool(name="ps_s", bufs=3, space="PSUM")
            ps_s = _ps_s_cm.__enter__()
            _ps_acc_cm = tc.tile_pool(name="ps_acc", bufs=3, space="PSUM")
            ps_acc = _ps_acc_cm.__enter__()

            # ---- resident inputs ----
            def load_head_weights(h):
                wq_s = whead.tile([128, 4 * DK], F32R, tag="wq", name=f"wq{h}")
                nc.sync.dma_start(
                    wq_s[:].rearrange("p (k m) -> p k m", k=4),
                    wq_d[h].rearrange("(k p) m -> p k m", p=128),
                )
                wk_s = whead.tile([128, 4 * DK], F32R, tag="wk", name=f"wk{h}")
                nc.sync.dma_start(
                    wk_s[:].rearrange("p (k m) -> p k m", k=4),
                    wk_d[h].rearrange("(k p) m -> p k m", p=128),
                )
                wv_s = whead.tile([128, 4 * DV], F32R, tag="wv", name=f"wv{h}")
                nc.sync.dma_start(
                    wv_s[:].rearrange("p (k m) -> p k m", k=4),
                    wv_d[h].rearrange("(k p) m -> p k m", p=128),
                )
                bq_s = whead.tile([128, 1], F32, tag="bq", name=f"bq{h}")
                nc.sync.dma_start(bq_s[:], bq_d[h : h + 1, :].rearrange("o p -> p o"))
                bk_s = whead.tile([128, 1], F32, tag="bk", name=f"bk{h}")
                nc.sync.dma_start(bk_s[:], bk_d[h : h + 1, :].rearrange("o p -> p o"))
                return wq_s, wk_s, wv_s, bq_s, bk_s

            qTs, kTs, vTs = [], [], []
            for name, dram, lst in (("q", qT_d, qTs), ("k", kT_d, kTs), ("v", vT_d, vTs)):
                for k in range(4):
                    t = const.tile([128, S], F32R, tag=f"{name}T{k}", name=f"{name}T{k}")
                    lst.append(t)
            # startup-ordered loads: each projection's weight right before
            # the input tensor it contracts with
            wq_s0 = whead.tile([128, 4 * DK], F32R, tag="wq", name="wq0")
            nc.sync.dma_start(
                wq_s0[:].rearrange("p (k m) -> p k m", k=4),
                wq_d[0].rearrange("(k p) m -> p k m", p=128),
            )
            nc.sync.dma_start(qTs[0][:, 0:512], qT_d[0:128, 0:512])
            nc.sync.dma_start(qTs[0][:, 512:1024], qT_d[0:128, 512:1024])
            for k in range(1, 4):
                nc.sync.dma_start(qTs[k][:], qT_d[128 * k : 128 * k + 128, :])
            wk_s0 = whead.tile([128, 4 * DK], F32R, tag="wk", name="wk0")
            nc.sync.dma_start(
                wk_s0[:].rearrange("p (k m) -> p k m", k=4),
                wk_d[0].rearrange("(k p) m -> p k m", p=128),
            )
            nc.sync.dma_start(kTs[0][:, 0:512], kT_d[0:128, 0:512])
            nc.sync.dma_start(kTs[0][:, 512:1024], kT_d[0:128, 512:1024])
            for k in range(1, 4):
                nc.sync.dma_start(kTs[k][:], kT_d[128 * k : 128 * k + 128, :])
            bq_s0 = whead.tile([128, 1], F32, tag="bq", name="bq0")
            nc.sync.dma_start(bq_s0[:], bq_d[0:1, :].rearrange("o p -> p o"))
            bk_s0 = whead.tile([128, 1], F32, tag="bk", name="bk0")
            nc.sync.dma_start(bk_s0[:], bk_d[0:1, :].rearrange("o p -> p o"))
            head1_weights = load_head_weights(1)
            wv_s0 = whead.tile([128, 4 * DV], F32R, tag="wv", name="wv0")
            nc.sync.dma_start(
                wv_s0[:].rearrange("p (k m) -> p k m", k=4),
                wv_d[0].rearrange("(k p) m -> p k m", p=128),
            )
            for k in range(4):
                nc.sync.dma_start(vTs[k][:], vT_d[128 * k : 128 * k + 128, :])
            head0_weights = (wq_s0, wk_s0, wv_s0, bq_s0, bk_s0)
            mask_s = const.tile([128, 896], F32, tag="maskstrip")
            nc.sync.dma_start(mask_s[:], mask_d[:])
            onescol = const.tile([128, 1], F32R, tag="onescol")
            nc.sync.dma_start(onescol[:], onescol_d[:])

            oT = [oTp.tile([128, S], F32R, tag=f"oT{i}", name=f"oT{i}") for i in range(16)]

            # ---- per-head projections + attention (software-pipelined:
            # head h+1's Q/K projections are emitted before head h's V
            # projection so the PE never queues behind vT-gated work) ----
            def proj_qk(h, weights):
                wq_s, wk_s, _, bq_s, bk_s = weights
                qpT = proj.tile([128, S], F32R, tag="qpT", name=f"qpT{h}")
                kpT = proj.tile([128, S], F32R, tag="kpT", name=f"kpT{h}")
                for dst, w_s, src, b_s in ((qpT, wq_s, qTs, bq_s), (kpT, wk_s, kTs, bk_s)):
                    for c in range(2):
                        p = ps_a.tile([128, 512], F32, tag="pa")
                        for k in range(4):
                            nc.tensor.matmul(
                                p[:],
                                w_s[:, 128 * k : 128 * k + 128],
                                src[k][:, 512 * c : 512 * c + 512],
                                start=(k == 0),
                                stop=(k == 3),
                            )
                        if c == 0:
                            nc.scalar.activation(
                                dst[:, 512 * c : 512 * c + 512], p[:], ACT.Identity,
                                bias=b_s[:],
                            )
                        else:
                            nc.vector.tensor_scalar_add(
                                dst[:, 512 * c : 512 * c + 512], p[:], b_s[:]
                            )
                return qpT, kpT

            def proj_v(h, weights):
                wv_s = weights[2]
                vp = proj.tile([128, 8 * DV], F32R, tag="vp", name=f"vp{h}")
                for i in range(8):
                    p = ps_a.tile([128, DV], F32, tag="pa")
                    for k in range(4):
                        nc.tensor.matmul(
                            p[:],
                            vTs[k][:, 128 * i : 128 * i + 128],
                            wv_s[:, DV * k : DV * k + DV],
                            start=(k == 0),
                            stop=(k == 3),
                        )
                    if i % 2 == 0:
                        nc.scalar.activation(
                            vp[:, DV * i : DV * i + DV], p[:], ACT.Copy
                        )
                    else:
                        nc.vector.tensor_copy(vp[:, DV * i : DV * i + DV], p[:])
                return vp

            def attn(h, qpT, kpT, vp):
                last = None
                # attention per 512-wide q-chunk
                for j in range(2):
                    n_t = 4 * (j + 1)
                    qlo = 512 * j
                    po = [
                        ps_acc.tile([128, 512], F32, tag="acc", name=f"po{vh}")
                        for vh in range(2)
                    ]
                    pr = ps_acc.tile([1, 512], F32, tag="acc")
                    for i in range(n_t):
                        # live column window: causality kills q < 128*r in
                        # this t-tile; round the window down to >=256 wide so
                        # f32r stays at full rate
                        r = i - 4 * j
                        wlo = 0 if r < 1 else min(128 * r, 256)
                        nw = 512 - wlo
                        psc = ps_s.tile([128, nw], F32, tag="ps", name=f"psc{i}")
                        nc.tensor.matmul(
                            psc[:],
                            kpT[:, 128 * i : 128 * i + 128],
                            qpT[:, qlo + wlo : qlo + 512],
                            start=True,
                            stop=True,
                        )
                        pt = ptp.tile([128, nw], F32R, tag="pt", name=f"pt{i}")
                        nc.scalar.activation(pt[:], psc[:], ACT.Exp, scale=SCALE)
                        if 0 <= r <= 2:
                            lo = 128 * r - wlo
                            nc.vector.tensor_mul(
                                pt[:, lo : lo + 128],
                                pt[:, lo : lo + 128],
                                mask_s[:, 384:512],
                            )
                        elif r == 3:
                            nc.vector.tensor_mul(
                                pt[:, 0:256],
                                pt[:, 0:256],
                                mask_s[:, 256:512],
                            )
                        for vh in range(2):
                            nc.tensor.matmul(
                                po[vh][:, wlo:512],
                                vp[:, DV * i + 128 * vh : DV * i + 128 * vh + 128],
                                pt[:],
                                start=(i == 0),
                                stop=(i == n_t - 1),
                                skip_group_check=True,
                            )
                        nc.tensor.matmul(
                            pr[:, wlo:512],
                            onescol[:],
                            pt[:],
                            start=(i == 0),
                            stop=(i == n_t - 1),
                            skip_group_check=True,
                        )
                    recip = recipp.tile([1, 512], F32, tag="recip")
                    nc.vector.reciprocal(recip[:], pr[:])
                    pbs = recipp.tile([128, 512], F32, tag="pbs")
                    nc.gpsimd.partition_broadcast(pbs[:], recip[:], 128)
                    for vh in range(2):
                        mm = nc.vector.tensor_mul(
                            oT[2 * h + vh][:, qlo : qlo + 512], po[vh][:], pbs[:]
                        )
                        last = mm
                return last

            weights = {0: head0_weights, 1: head1_weights}
            for h in range(H):
                if h not in weights:
                    weights[h] = load_head_weights(h)
                qpT_h, kpT_h = proj_qk(h, weights[h])
                vp_h = proj_v(h, weights[h])
                last_attn = attn(h, qpT_h, kpT_h, vp_h)

            # ---- output projection: outT[m, s] ----
            # kk outer so each wo tile is consumed in one burst (4 wop slots
            # suffice); 8 psum accumulators live, gated behind the end of
            # attention so PSUM banks never overcommit.
            _pools8 = [ps_a, ps_a, ps_s, ps_s, ps_s, ps_acc, ps_acc, ps_acc]
            _tags8 = ["pa", "pa", "ps", "ps", "ps", "acc", "acc", "acc"]
            po8 = [
                _pools8[g].tile([128, 512], F32, tag=_tags8[g], name=f"pout{g}")
                for g in range(8)
            ]
            # phase A: kk-outer over first half of the contraction
            wo_tiles = {}
            for kk in range(8):
                w = wop.tile([128, D], F32R, tag="wo", name=f"wo{kk}")
                nc.sync.dma_start(w[:], wo_d[128 * kk : 128 * kk + 128, :])
                for g in range(8):
                    m, c = divmod(g, 2)
                    mm = nc.tensor.matmul(
                        po8[g][:],
                        w[:, 128 * m : 128 * m + 128],
                        oT[kk][:, 512 * c : 512 * c + 512],
                        start=(kk == 0),
                        stop=False,
                    )
                    if kk == 0:
                        add_dep_helper(
                            mm.ins,
                            last_attn.ins,
                            sync=False,
                            reason="out-proj psum after attention psum freed",
                        )
            # phase B: group-major so early groups finish, evict and DMA out
            # while later groups still accumulate
            for kk in range(8, 16):
                w = wop.tile([128, D], F32R, tag="wo", name=f"wo{kk}")
                nc.sync.dma_start(w[:], wo_d[128 * kk : 128 * kk + 128, :])
                wo_tiles[kk] = w
            for g in range(8):
                m, c = divmod(g, 2)
                for kk in range(8, 16):
                    nc.tensor.matmul(
                        po8[g][:],
                        wo_tiles[kk][:, 128 * m : 128 * m + 128],
                        oT[kk][:, 512 * c : 512 * c + 512],
                        start=False,
                        stop=(kk == 15),
                    )
                st = outst.tile([128, 512], F32, tag="outst")
                nc.scalar.activation(st[:], po8[g][:], ACT.Copy)
                nc.sync.dma_start(
                    outT_d[128 * m : 128 * m + 128, 512 * c : 512 * c + 512], st[:]
                )
            _ps_acc_cm.__exit__(None, None, None)
            _ps_s_cm.__exit__(None, None, None)
            attn_psum.__exit__(None, None, None)

    nc.compile()
    return nc


def _prep(Q, K, V, padding_mask, sequence_mask, Wq, bq, Wk, bk, Wv, bv, Wo, bo):
    assert padding_mask.min() == 1, "kernel assumes all-ones padding mask"
    seq = np.asarray(sequence_mask)
    tril = seq[0:128, 0:128].T.astype(np.float32)
    maskstrip = np.concatenate(
        [np.zeros((128, 384), np.float32), tril, np.ones((128, 384), np.float32)],
        axis=1,
    )
    for j in range(2):
        for i in range(4 * j, 4 * j + 4):
            r = i - 4 * j
            blk = seq[
                512 * j : 512 * j + 512, 128 * i : 128 * i + 128
            ].T.astype(np.float32)
            assert np.array_equal(
                blk, maskstrip[:, 384 - 128 * r : 896 - 128 * r]
            ), "kernel assumes causal sequence mask"
        for i in range(4 * j):
            assert seq[512 * j : 512 * j + 512, 128 * i : 128 * i + 128].min() == 1
    c = np.ascontiguousarray
    shared = {
        "wq": c(Wq.astype(np.float32)),
        "wk": c(Wk.astype(np.float32)),
        "wv": c(Wv.astype(np.float32)),
        "wo": c(Wo.astype(np.float32)),
        "bq": c(bq.astype(np.float32)),
        "bk": c(bk.astype(np.float32)),
        "maskstrip": maskstrip,
        "onescol": np.ones((128, 1), np.float32),
    }
    in_maps = []
    for b in range(B):
        m = dict(shared)
        m["qT"] = c(np.asarray(Q[b]).T.astype(np.float32))
        m["kT"] = c(np.asarray(K[b]).T.astype(np.float32))
        m["vT"] = c(np.asarray(V[b]).T.astype(np.float32))
        in_maps.append(m)
    bo_eff = (
        np.asarray(bo, np.float32)
        + np.asarray(bv, np.float32).reshape(H * DV) @ np.asarray(Wo, np.float32)
    ).astype(np.float32)
    return in_maps, bo_eff


def kernel(Q, K, V, padding_mask, sequence_mask, Wq, bq, Wk, bk, Wv, bv, Wo, bo):
    if "nc" not in _CACHE:
        _CACHE["nc"] = build()
    nc = _CACHE["nc"]
    in_maps, bo_eff = _prep(
        Q, K, V, padding_mask, sequence_mask, Wq, bq, Wk, bk, Wv, bv, Wo, bo
    )
    res = run_bass_kernel_spmd(nc, in_maps, core_ids=list(range(B)))
    out = np.empty((B, S, D), np.float32)
    for b in range(B):
        out[b] = res.results[b]["outT"].T + bo_eff
    return out


# revision 46
# speedup vs baseline: 63.2117x; 1.0003x over previous
"""Trainium2 Bass kernel for 8-head causal MultiHeadAttention.

Problem (hardcoded): B=8, S=1024, d_model=512, H=8, d_k=128, d_v=256,
causal sequence mask, all-ones padding mask, fp32.

Strategy:
  - Batch-parallel across the 8 NeuronCores (1 batch element per core).
  - All matmuls in float32r (TF32-like fp32 @ 4x fp32 rate; ~13 mantissa
    bits) with every matmul free dim >= 256 for the full 1 cycle/row rate.
  - Scores are computed TRANSPOSED (S^T[t, q]) so the P@V contraction needs
    no transposes of the attention matrix; softmax denominators via a
    ones-vector matmul; normalization applied to O^T after the PV matmul;
    reciprocal broadcast across partitions with a rank-1 PE matmul.
  - Causality handled structurally (lower-triangular t-tiles only) plus
    0/1 mask multiplies on diagonal-band blocks.
  - Host side: transposes Q/K/V per batch element (so the kernel DMAs are
    contiguous), folds bv through softmax (rows sum to 1) and bo into a
    single host-side bias add, and transposes the per-core out^T back.
"""

import numpy as np

import concourse.bacc as bacc
import concourse.mybir as mybir
from concourse import tile
from concourse.bass_utils import run_bass_kernel_spmd
from concourse.tile_rust import add_dep_helper

B, S, D, H, DK, DV = 8, 1024, 512, 8, 128, 256
F32 = mybir.dt.float32
F32R = mybir.dt.float32r
ACT = mybir.ActivationFunctionType
SCALE = float(np.float32(1.0) / np.sqrt(np.float32(DK)).astype(np.float32))

_CACHE = {}


def build():
    nc = bacc.Bacc(trn_type="TRN2", target_bir_lowering=False, debug=False)

    qT_d = nc.dram_tensor("qT", [D, S], F32R, kind="ExternalInput").ap()
    kT_d = nc.dram_tensor("kT", [D, S], F32R, kind="ExternalInput").ap()
    vT_d = nc.dram_tensor("vT", [D, S], F32R, kind="ExternalInput").ap()
    wq_d = nc.dram_tensor("wq", [H, D, DK], F32R, kind="ExternalInput").ap()
    wk_d = nc.dram_tensor("wk", [H, D, DK], F32R, kind="ExternalInput").ap()
    wv_d = nc.dram_tensor("wv", [H, D, DV], F32R, kind="ExternalInput").ap()
    wo_d = nc.dram_tensor("wo", [H * DV, D], F32R, kind="ExternalInput").ap()
    bq_d = nc.dram_tensor("bq", [H, DK], F32, kind="ExternalInput").ap()
    bk_d = nc.dram_tensor("bk", [H, DK], F32, kind="ExternalInput").ap()
    mask_d = nc.dram_tensor("maskstrip", [128, 896], F32, kind="ExternalInput").ap()
    onescol_d = nc.dram_tensor("onescol", [128, 1], F32R, kind="ExternalInput").ap()
    outT_d = nc.dram_tensor("outT", [D, S], F32, kind="ExternalOutput").ap()

    with tile.TileContext(nc) as tc:
        with (
            tc.tile_pool(name="const", bufs=1) as const,
            tc.tile_pool(name="oTp", bufs=1) as oTp,
            tc.tile_pool(name="whead", bufs=2) as whead,
            tc.tile_pool(name="proj", bufs=2) as proj,
            tc.tile_pool(name="ptp", bufs=6) as ptp,
            tc.tile_pool(name="wop", bufs=8) as wop,
            tc.tile_pool(name="outst", bufs=2) as outst,
            tc.tile_pool(name="recipp", bufs=3) as recipp,
        ):
            attn_psum = tc.tile_pool(name="ps_a", bufs=2, space="PSUM")
            ps_a = attn_psum.__enter__()
            _ps_s_cm = tc.tile_pool(name="ps_s", bufs=3, space="PSUM")
            ps_s = _ps_s_cm.__enter__()
            _ps_acc_cm = tc.tile_pool(name="ps_acc", bufs=3, space="PSUM")
            ps_acc = _ps_acc_cm.__enter__()

            # ---- resident inputs ----
            def load_head_weights(h):
                wq_s = whead.tile([128, 4 * DK], F32R, tag="wq", name=f"wq{h}")
                nc.sync.dma_start(
                    wq_s[:].rearrange("p (k m) -> p k m", k=4),
                    wq_d[h].rearrange("(k p) m -> p k m", p=128),
                )
                wk_s = whead.tile([128, 4 * DK], F32R, tag="wk", name=f"wk{h}")
                nc.sync.dma_start(
                    wk_s[:].rearrange("p (k m) -> p k m", k=4),
                    wk_d[h].rearrange("(k p) m -> p k m", p=128),
                )
                wv_s = whead.tile([128, 4 * DV], F32R, tag="wv", name=f"wv{h}")
                nc.sync.dma_start(
                    wv_s[:].rearrange("p (k m) -> p k m", k=4),
                    wv_d[h].rearrange("(k p) m -> p k m", p=128),
                )
                bq_s = whead.tile([128, 1], F32, tag="bq", name=f"bq{h}")
                nc.sync.dma_start(bq_s[:], bq_d[h : h + 1, :].rearrange("o p -> p o"))
                bk_s = whead.tile([128, 1], F32, tag="bk", name=f"bk{h}")
                nc.sync.dma_start(bk_s[:], bk_d[h : h + 1, :].rearrange("o p -> p o"))
                return wq_s, wk_s, wv_s, bq_s, bk_s

            qTs, kTs, vTs = [], [], []
            for name, dram, lst in (("q", qT_d, qTs), ("k", kT_d, kTs), ("v", vT_d, vTs)):
                for k in range(4):
                    t = const.tile([128, S], F32R, tag=f"{name}T{k}", name=f"{name}T{k}")
                    lst.append(t)
            # startup-ordered loads: each projection's weight right before
            # the input tensor it contracts with
            wq_s0 = whead.tile([128, 4 * DK], F32R, tag="wq", name="wq0")
            nc.sync.dma_start(
                wq_s0[:].rearrange("p (k m) -> p k m", k=4),
                wq_d[0].rearrange("(k p) m -> p k m", p=128),
            )
            nc.sync.dma_start(qTs[0][:, 0:512], qT_d[0:128, 0:512])
            nc.sync.dma_start(qTs[0][:, 512:1024], qT_d[0:128, 512:1024])
            for k in range(1, 4):
                nc.sync.dma_start(qTs[k][:], qT_d[128 * k : 128 * k + 128, :])
            wk_s0 = whead.tile([128, 4 * DK], F32R, tag="wk", name="wk0")
            nc.sync.dma_start(
                wk_s0[:].rearrange("p (k m) -> p k m", k=4),
                wk_d[0].rearrange("(k p) m -> p k m", p=128),
            )
            nc.sync.dma_start(kTs[0][:, 0:512], kT_d[0:128, 0:512])
            nc.sync.dma_start(kTs[0][:, 512:1024], kT_d[0:128, 512:1024])
            for k in range(1, 4):
                nc.sync.dma_start(kTs[k][:], kT_d[128 * k : 128 * k + 128, :])
            bq_s0 = whead.tile([128, 1], F32, tag="bq", name="bq0")
            nc.sync.dma_start(bq_s0[:], bq_d[0:1, :].rearrange("o p -> p o"))
            bk_s0 = whead.tile([128, 1], F32, tag="bk", name="bk0")
            nc.sync.dma_start(bk_s0[:], bk_d[0:1, :].rearrange("o p -> p o"))
            head1_weights = load_head_weights(1)
            wv_s0 = whead.tile([128, 4 * DV], F32R, tag="wv", name="wv0")
            nc.sync.dma_start(
                wv_s0[:].rearrange("p (k m) -> p k m", k=4),
                wv_d[0].rearrange("(k p) m -> p k m", p=128),
            )
            for k in range(4):
                nc.sync.dma_start(vTs[k][:], vT_d[128 * k : 128 * k + 128, :])
            head0_weights = (wq_s0, wk_s0, wv_s0, bq_s0, bk_s0)
            mask_s = const.tile([128, 896], F32, tag="maskstrip")
            nc.sync.dma_start(mask_s[:], mask_d[:])
            onescol = const.tile([128, 1], F32R, tag="onescol")
            nc.sync.dma_start(onescol[:], onescol_d[:])

            oT = [oTp.tile([128, S], F32R, tag=f"oT{i}", name=f"oT{i}") for i in range(16)]

            # ---- per-head projections + attention (software-pipelined:
            # head h+1's Q/K projections are emitted before head h's V
            # projection so the PE never queues behind vT-gated work) ----
            def proj_qk(h, weights):
                wq_s, wk_s, _, bq_s, bk_s = weights
                qpT = proj.tile([128, S], F32R, tag="qpT", name=f"qpT{h}")
                kpT = proj.tile([128, S], F32R, tag="kpT", name=f"kpT{h}")
                for dst, w_s, src, b_s in ((qpT, wq_s, qTs, bq_s), (kpT, wk_s, kTs, bk_s)):
                    for c in range(2):
                        p = ps_a.tile([128, 512], F32, tag="pa")
                        for k in range(4):
                            nc.tensor.matmul(
                                p[:],
                                w_s[:, 128 * k : 128 * k + 128],
                                src[k][:, 512 * c : 512 * c + 512],
                                start=(k == 0),
                                stop=(k == 3),
                            )
                        if c == 0:
                            nc.scalar.activation(
                                dst[:, 512 * c : 512 * c + 512], p[:], ACT.Identity,
                                bias=b_s[:],
                            )
                        else:
                            nc.vector.tensor_scalar_add(
                                dst[:, 512 * c : 512 * c + 512], p[:], b_s[:]
                            )
                return qpT, kpT

            def proj_v(h, weights):
                wv_s = weights[2]
                vp = proj.tile([128, 8 * DV], F32R, tag="vp", name=f"vp{h}")
                for i in range(8):
                    p = ps_a.tile([128, DV], F32, tag="pa")
                    for k in range(4):
                        nc.tensor.matmul(
                            p[:],
                            vTs[k][:, 128 * i : 128 * i + 128],
                            wv_s[:, DV * k : DV * k + DV],
                            start=(k == 0),
                            stop=(k == 3),
                        )
                    if i % 2 == 0:
                        nc.scalar.activation(
                            vp[:, DV * i : DV * i + DV], p[:], ACT.Copy
                        )
                    else:
                        nc.vector.tensor_copy(vp[:, DV * i : DV * i + DV], p[:])
                return vp

            def attn(h, qpT, kpT, vp):
                last = None
                # attention per 512-wide q-chunk
                for j in range(2):
                    n_t = 4 * (j + 1)
                    qlo = 512 * j
                    po = [
                        ps_acc.tile([128, 512], F32, tag="acc", name=f"po{vh}")
                        for vh in range(2)
                    ]
                    pr = ps_acc.tile([1, 512], F32, tag="acc")
                    for i in range(n_t):
                        # live column window: causality kills q < 128*r in
                        # this t-tile; round the window down to >=256 wide so
                        # f32r stays at full rate
                        r = i - 4 * j
                        wlo = 0 if r < 1 else min(128 * r, 256)
                        nw = 512 - wlo
                        psc = ps_s.tile([128, nw], F32, tag="ps", name=f"psc{i}")
                        nc.tensor.matmul(
                            psc[:],
                            kpT[:, 128 * i : 128 * i + 128],
                            qpT[:, qlo + wlo : qlo + 512],
                            start=True,
                            stop=True,
                        )
                        pt = ptp.tile([128, nw], F32R, tag="pt", name=f"pt{i}")
                        nc.scalar.activation(pt[:], psc[:], ACT.Exp, scale=SCALE)
                        if 0 <= r <= 2:
                            lo = 128 * r - wlo
                            nc.vector.tensor_mul(
                                pt[:, lo : lo + 128],
                                pt[:, lo : lo + 128],
                                mask_s[:, 384:512],
                            )
                        elif r == 3:
                            nc.vector.tensor_mul(
                                pt[:, 0:256],
                                pt[:, 0:256],
                                mask_s[:, 256:512],
                            )
                        for vh in range(2):
                            nc.tensor.matmul(
                                po[vh][:, wlo:512],
                                vp[:, DV * i + 128 * vh : DV * i + 128 * vh + 128],
                                pt[:],
                                start=(i == 0),
                                stop=(i == n_t - 1),
                                skip_group_check=True,
                            )
                        nc.tensor.matmul(
                            pr[:, wlo:512],
                            onescol[:],
                            pt[:],
                            start=(i == 0),
                            stop=(i == n_t - 1),
                            skip_group_check=True,
                        )
                    recip = recipp.tile([1, 512], F32, tag="recip")
                    nc.vector.reciprocal(recip[:], pr[:])
                    pbs = recipp.tile([128, 512], F32, tag="pbs")
                    nc.gpsimd.partition_broadcast(pbs[:], recip[:], 128)
                    for vh in range(2):
                        mm = nc.vector.tensor_mul(
                            oT[2 * h + vh][:, qlo : qlo + 512], po[vh][:], pbs[:]
                        )
                        last = mm
                return last

            weights = {0: head0_weights, 1: head1_weights}
            for h in range(H):
                if h not in weights:
                    weights[h] = load_head_weights(h)
                qpT_h, kpT_h = proj_qk(h, weights[h])
                vp_h = proj_v(h, weights[h])
                last_attn = attn(h, qpT_h, kpT_h, vp_h)

            # ---- output projection: outT[m, s] ----
            # kk outer so each wo tile is consumed in one burst (4 wop slots
            # suffice); 8 psum accumulators live, gated behind the end of
            # attention so PSUM banks never overcommit.
            _pools8 = [ps_a, ps_a, ps_s, ps_s, ps_s, ps_acc, ps_acc, ps_acc]
            _tags8 = ["pa", "pa", "ps", "ps", "ps", "acc", "acc", "acc"]
            po8 = [
                _pools8[g].tile([128, 512], F32, tag=_tags8[g], name=f"pout{g}")
                for g in range(8)
            ]
            # phase A: kk-outer over first half of the contraction
            wo_tiles = {}
            for kk in range(8):
                w = wop.tile([128, D], F32R, tag="wo", name=f"wo{kk}")
                nc.sync.dma_start(w[:], wo_d[128 * kk : 128 * kk + 128, :])
                for g in range(8):
                    m, c = divmod(g, 2)
                    mm = nc.tensor.matmul(
                        po8[g][:],
                        w[:, 128 * m : 128 * m + 128],
                        oT[kk][:, 512 * c : 512 * c + 512],
                        start=(kk == 0),
                        stop=False,
                    )
                    if kk == 0:
                        add_dep_helper(
                            mm.ins,
                            last_attn.ins,
                            sync=False,
                            reason="out-proj psum after attention psum freed",
                        )
            # phase B: group-major so early groups finish, evict and DMA out
            # while later groups still accumulate
            for kk in range(8, 16):
                w = wop.tile([128, D], F32R, tag="wo", name=f"wo{kk}")
                nc.sync.dma_start(w[:], wo_d[128 * kk : 128 * kk + 128, :])
                wo_tiles[kk] = w
            for g in range(8):
                m, c = divmod(g, 2)
                for kk in range(8, 16):
                    nc.tensor.matmul(
                        po8[g][:],
                        wo_tiles[kk][:, 128 * m : 128 * m + 128],
                        oT[kk][:, 512 * c : 512 * c + 512],
                        start=False,
                        stop=(kk == 15),
                    )
                st = outst.tile([128, 512], F32, tag="outst")
                nc.scalar.activation(st[:], po8[g][:], ACT.Copy)
                nc.sync.dma_start(
                    outT_d[128 * m : 128 * m + 128, 512 * c : 512 * c + 512], st[:]
                )
            _ps_acc_cm.__exit__(None, None, None)
            _ps_s_cm.__exit__(None, None, None)
            attn_psum.__exit__(None, None, None)

    nc.compile()
    return nc


def _prep(Q, K, V, padding_mask, sequence_mask, Wq, bq, Wk, bk, Wv, bv, Wo, bo):
    assert padding_mask.min() == 1, "kernel assumes all-ones padding mask"
    seq = np.asarray(sequence_mask)
    tril = seq[0:128, 0:128].T.astype(np.float32)
    maskstrip = np.concatenate(
        [np.zeros((128, 384), np.float32), tril, np.ones((128, 384), np.float32)],
        axis=1,
    )
    for j in range(2):
        for i in range(4 * j, 4 * j + 4):
            r = i - 4 * j
            blk = seq[
                512 * j : 512 * j + 512, 128 * i : 128 * i + 128
            ].T.astype(np.float32)
            assert np.array_equal(
                blk, maskstrip[:, 384 - 128 * r : 896 - 128 * r]
            ), "kernel assumes causal sequence mask"
        for i in range(4 * j):
            assert seq[512 * j : 512 * j + 512, 128 * i : 128 * i + 128].min() == 1
    c = np.ascontiguousarray
    shared = {
        "wq": c(Wq.astype(np.float32)),
        "wk": c(Wk.astype(np.float32)),
        "wv": c(Wv.astype(np.float32)),
        "wo": c(Wo.astype(np.float32)),
        "bq": c(bq.astype(np.float32)),
        "bk": c(bk.astype(np.float32)),
        "maskstrip": maskstrip,
        "onescol": np.ones((128, 1), np.float32),
    }
    in_maps = []
    for b in range(B):
        m = dict(shared)
        m["qT"] = c(np.asarray(Q[b]).T.astype(np.float32))
        m["kT"] = c(np.asarray(K[b]).T.astype(np.float32))
        m["vT"] = c(np.asarray(V[b]).T.astype(np.float32))
        in_maps.append(m)
    bo_eff = (
        np.asarray(bo, np.float32)
        + np.asarray(bv, np.float32).reshape(H * DV) @ np.asarray(Wo, np.float32)
    ).astype(np.float32)
    return in_maps, bo_eff


def kernel(Q, K, V, padding_mask, sequence_mask, Wq, bq, Wk, bk, Wv, bv, Wo, bo):
    if "nc" not in _CACHE:
        _CACHE["nc"] = build()
    nc = _CACHE["nc"]
    in_maps, bo_eff = _prep(
        Q, K, V, padding_mask, sequence_mask, Wq, bq, Wk, bk, Wv, bv, Wo, bo
    )
    res = run_bass_kernel_spmd(nc, in_maps, core_ids=list(range(B)))
    out = np.empty((B, S, D), np.float32)
    for b in range(B):
        out[b] = res.results[b]["outT"].T + bo_eff
    return out


# revision 58
# speedup vs baseline: 63.2656x; 1.0009x over previous
"""Trainium2 Bass kernel for 8-head causal MultiHeadAttention.

Problem (hardcoded): B=8, S=1024, d_model=512, H=8, d_k=128, d_v=256,
causal sequence mask, all-ones padding mask, fp32.

Strategy:
  - Batch-parallel across the 8 NeuronCores (1 batch element per core).
  - All matmuls in float32r (TF32-like fp32 @ 4x fp32 rate; ~13 mantissa
    bits) with every matmul free dim >= 256 for the full 1 cycle/row rate.
  - Scores are computed TRANSPOSED (S^T[t, q]) so the P@V contraction needs
    no transposes of the attention matrix; softmax denominators via a
    ones-vector matmul; normalization applied to O^T after the PV matmul;
    reciprocal broadcast across partitions with a rank-1 PE matmul.
  - Causality handled structurally (lower-triangular t-tiles only) plus
    0/1 mask multiplies on diagonal-band blocks.
  - Host side: transposes Q/K/V per batch element (so the kernel DMAs are
    contiguous), folds bv through softmax (rows sum to 1) and bo into a
    single host-side bias add, and transposes the per-core out^T back.
"""

import numpy as np

import concourse.bacc as bacc
import concourse.mybir as mybir
from concourse import tile
from concourse.bass_utils import run_bass_kernel_spmd
from concourse.tile_rust import add_dep_helper

B, S, D, H, DK, DV = 8, 1024, 512, 8, 128, 256
F32 = mybir.dt.float32
F32R = mybir.dt.float32r
ACT = mybir.ActivationFunctionType
SCALE = float(np.float32(1.0) / np.sqrt(np.float32(DK)).astype(np.float32))

_CACHE = {}


def build():
    nc = bacc.Bacc(trn_type="TRN2", target_bir_lowering=False, debug=False)

    qT_d = nc.dram_tensor("qT", [D, S], F32R, kind="ExternalInput").ap()
    kT_d = nc.dram_tensor("kT", [D, S], F32R, kind="ExternalInput").ap()
    vT_d = nc.dram_tensor("vT", [D, S], F32R, kind="ExternalInput").ap()
    wq_d = nc.dram_tensor("wq", [H, D, DK], F32R, kind="ExternalInput").ap()
    wk_d = nc.dram_tensor("wk", [H, D, DK], F32R, kind="ExternalInput").ap()
    wv_d = nc.dram_tensor("wv", [H, D, DV], F32R, kind="ExternalInput").ap()
    wo_d = nc.dram_tensor("wo", [H * DV, D], F32R, kind="ExternalInput").ap()
    bq_d = nc.dram_tensor("bq", [H, DK], F32, kind="ExternalInput").ap()
    bk_d = nc.dram_tensor("bk", [H, DK], F32, kind="ExternalInput").ap()
    mask_d = nc.dram_tensor("maskstrip", [128, 640], F32, kind="ExternalInput").ap()
    onescol_d = nc.dram_tensor("onescol", [128, 1], F32R, kind="ExternalInput").ap()
    outT_d = nc.dram_tensor("outT", [D, S], F32, kind="ExternalOutput").ap()

    with tile.TileContext(nc) as tc:
        with (
            tc.tile_pool(name="const", bufs=1) as const,
            tc.tile_pool(name="oTp", bufs=1) as oTp,
            tc.tile_pool(name="whead", bufs=2) as whead,
            tc.tile_pool(name="proj", bufs=2) as proj,
            tc.tile_pool(name="ptp", bufs=6) as ptp,
            tc.tile_pool(name="wop", bufs=8) as wop,
            tc.tile_pool(name="outst", bufs=2) as outst,
            tc.tile_pool(name="recipp", bufs=3) as recipp,
        ):
            attn_psum = tc.tile_pool(name="ps_a", bufs=2, space="PSUM")
            ps_a = attn_psum.__enter__()
            _ps_s_cm = tc.tile_pool(name="ps_s", bufs=3, space="PSUM")
            ps_s = _ps_s_cm.__enter__()
            _ps_acc_cm = tc.tile_pool(name="ps_acc", bufs=3, space="PSUM")
            ps_acc = _ps_acc_cm.__enter__()

            # ---- resident inputs ----
            def load_head_weights(h):
                wq_s = whead.tile([128, 4 * DK], F32R, tag="wq", name=f"wq{h}")
                nc.sync.dma_start(
                    wq_s[:].rearrange("p (k m) -> p k m", k=4),
                    wq_d[h].rearrange("(k p) m -> p k m", p=128),
                )
                wk_s = whead.tile([128, 4 * DK], F32R, tag="wk", name=f"wk{h}")
                nc.sync.dma_start(
                    wk_s[:].rearrange("p (k m) -> p k m", k=4),
                    wk_d[h].rearrange("(k p) m -> p k m", p=128),
                )
                wv_s = whead.tile([128, 4 * DV], F32R, tag="wv", name=f"wv{h}")
                nc.sync.dma_start(
                    wv_s[:].rearrange("p (k m) -> p k m", k=4),
                    wv_d[h].rearrange("(k p) m -> p k m", p=128),
                )
                bq_s = whead.tile([128, 1], F32, tag="bq", name=f"bq{h}")
                nc.sync.dma_start(bq_s[:], bq_d[h : h + 1, :].rearrange("o p -> p o"))
                bk_s = whead.tile([128, 1], F32, tag="bk", name=f"bk{h}")
                nc.sync.dma_start(bk_s[:], bk_d[h : h + 1, :].rearrange("o p -> p o"))
                return wq_s, wk_s, wv_s, bq_s, bk_s

            qTs, kTs, vTs = [], [], []
            for name, dram, lst in (("q", qT_d, qTs), ("k", kT_d, kTs), ("v", vT_d, vTs)):
                for k in range(4):
                    t = const.tile([128, S], F32R, tag=f"{name}T{k}", name=f"{name}T{k}")
                    lst.append(t)
            # startup-ordered loads: each projection's weight right before
            # the input tensor it contracts with
            wq_s0 = whead.tile([128, 4 * DK], F32R, tag="wq", name="wq0")
            nc.sync.dma_start(
                wq_s0[:].rearrange("p (k m) -> p k m", k=4),
                wq_d[0].rearrange("(k p) m -> p k m", p=128),
            )
            nc.sync.dma_start(qTs[0][:, 0:512], qT_d[0:128, 0:512])
            nc.sync.dma_start(qTs[0][:, 512:1024], qT_d[0:128, 512:1024])
            for k in range(1, 4):
                nc.sync.dma_start(qTs[k][:], qT_d[128 * k : 128 * k + 128, :])
            wk_s0 = whead.tile([128, 4 * DK], F32R, tag="wk", name="wk0")
            nc.sync.dma_start(
                wk_s0[:].rearrange("p (k m) -> p k m", k=4),
                wk_d[0].rearrange("(k p) m -> p k m", p=128),
            )
            nc.sync.dma_start(kTs[0][:, 0:512], kT_d[0:128, 0:512])
            nc.sync.dma_start(kTs[0][:, 512:1024], kT_d[0:128, 512:1024])
            for k in range(1, 4):
                nc.sync.dma_start(kTs[k][:], kT_d[128 * k : 128 * k + 128, :])
            bq_s0 = whead.tile([128, 1], F32, tag="bq", name="bq0")
            nc.sync.dma_start(bq_s0[:], bq_d[0:1, :].rearrange("o p -> p o"))
            bk_s0 = whead.tile([128, 1], F32, tag="bk", name="bk0")
            nc.sync.dma_start(bk_s0[:], bk_d[0:1, :].rearrange("o p -> p o"))
            head1_weights = load_head_weights(1)
            wv_s0 = whead.tile([128, 4 * DV], F32R, tag="wv", name="wv0")
            nc.sync.dma_start(
                wv_s0[:].rearrange("p (k m) -> p k m", k=4),
                wv_d[0].rearrange("(k p) m -> p k m", p=128),
            )
            for k in range(4):
                nc.sync.dma_start(vTs[k][:], vT_d[128 * k : 128 * k + 128, :])
            head0_weights = (wq_s0, wk_s0, wv_s0, bq_s0, bk_s0)
            mask_s = const.tile([128, 640], F32, tag="maskstrip")
            nc.sync.dma_start(mask_s[:], mask_d[:])
            onescol = const.tile([128, 1], F32R, tag="onescol")
            nc.sync.dma_start(onescol[:], onescol_d[:])

            oT = [oTp.tile([128, S], F32R, tag=f"oT{i}", name=f"oT{i}") for i in range(16)]

            # ---- per-head projections + attention (software-pipelined:
            # head h+1's Q/K projections are emitted before head h's V
            # projection so the PE never queues behind vT-gated work) ----
            def proj_qk(h, weights):
                wq_s, wk_s, _, bq_s, bk_s = weights
                qpT = proj.tile([128, S], F32R, tag="qpT", name=f"qpT{h}")
                kpT = proj.tile([128, S], F32R, tag="kpT", name=f"kpT{h}")
                for dst, w_s, src, b_s in ((qpT, wq_s, qTs, bq_s), (kpT, wk_s, kTs, bk_s)):
                    for c in range(2):
                        p = ps_a.tile([128, 512], F32, tag="pa")
                        for k in range(4):
                            nc.tensor.matmul(
                                p[:],
                                w_s[:, 128 * k : 128 * k + 128],
                                src[k][:, 512 * c : 512 * c + 512],
                                start=(k == 0),
                                stop=(k == 3),
                            )
                        if c == 0:
                            nc.scalar.activation(
                                dst[:, 512 * c : 512 * c + 512], p[:], ACT.Identity,
                                bias=b_s[:],
                            )
                        else:
                            nc.vector.tensor_scalar_add(
                                dst[:, 512 * c : 512 * c + 512], p[:], b_s[:]
                            )
                return qpT, kpT

            def proj_v(h, weights):
                wv_s = weights[2]
                vp = proj.tile([128, 8 * DV], F32R, tag="vp", name=f"vp{h}")
                for i in range(8):
                    p = ps_a.tile([128, DV], F32, tag="pa")
                    for k in range(4):
                        nc.tensor.matmul(
                            p[:],
                            vTs[k][:, 128 * i : 128 * i + 128],
                            wv_s[:, DV * k : DV * k + DV],
                            start=(k == 0),
                            stop=(k == 3),
                        )
                    if i % 2 == 0:
                        nc.scalar.activation(
                            vp[:, DV * i : DV * i + DV], p[:], ACT.Copy
                        )
                    else:
                        nc.vector.tensor_copy(vp[:, DV * i : DV * i + DV], p[:])
                return vp

            def attn(h, qpT, kpT, vp):
                last = None
                # attention per 512-wide q-chunk
                for j in range(2):
                    n_t = 4 * (j + 1)
                    qlo = 512 * j
                    po = [
                        ps_acc.tile([128, 512], F32, tag="acc", name=f"po{vh}")
                        for vh in range(2)
                    ]
                    pr = ps_acc.tile([1, 512], F32, tag="acc")
                    for i in range(n_t):
                        # live column window: causality kills q < 128*r in
                        # this t-tile; round the window down to >=256 wide so
                        # f32r stays at full rate
                        r = i - 4 * j
                        wlo = 0 if r < 1 else min(128 * r, 256)
                        nw = 512 - wlo
                        psc = ps_s.tile([128, nw], F32, tag="ps", name=f"psc{i}")
                        nc.tensor.matmul(
                            psc[:],
                            kpT[:, 128 * i : 128 * i + 128],
                            qpT[:, qlo + wlo : qlo + 512],
                            start=True,
                            stop=True,
                        )
                        pt = ptp.tile([128, nw], F32R, tag="pt", name=f"pt{i}")
                        nc.scalar.activation(pt[:], psc[:], ACT.Exp, scale=SCALE)
                        if 0 <= r <= 2:
                            lo = 128 * r - wlo
                            nc.vector.tensor_mul(
                                pt[:, lo : lo + 128],
                                pt[:, lo : lo + 128],
                                mask_s[:, 128:256],
                            )
                        elif r == 3:
                            nc.vector.tensor_mul(
                                pt[:, 0:256],
                                pt[:, 0:256],
                                mask_s[:, 0:256],
                            )
                        for vh in range(2):
                            nc.tensor.matmul(
                                po[vh][:, wlo:512],
                                vp[:, DV * i + 128 * vh : DV * i + 128 * vh + 128],
                                pt[:],
                                start=(i == 0),
                                stop=(i == n_t - 1),
                                skip_group_check=True,
                            )
                        nc.tensor.matmul(
                            pr[:, wlo:512],
                            onescol[:],
                            pt[:],
                            start=(i == 0),
                            stop=(i == n_t - 1),
                            skip_group_check=True,
                        )
                    recip = recipp.tile([1, 512], F32, tag="recip")
                    nc.vector.reciprocal(recip[:], pr[:])
                    pbs = recipp.tile([128, 512], F32, tag="pbs")
                    nc.gpsimd.partition_broadcast(pbs[:], recip[:], 128)
                    for vh in range(2):
                        mm = nc.vector.tensor_mul(
                            oT[2 * h + vh][:, qlo : qlo + 512], po[vh][:], pbs[:]
                        )
                        last = mm
                return last

            weights = {0: head0_weights, 1: head1_weights}
            for h in range(H):
                if h not in weights:
                    weights[h] = load_head_weights(h)
                qpT_h, kpT_h = proj_qk(h, weights[h])
                vp_h = proj_v(h, weights[h])
                last_attn = attn(h, qpT_h, kpT_h, vp_h)

            # ---- output projection: outT[m, s] ----
            # kk outer so each wo tile is consumed in one burst (4 wop slots
            # suffice); 8 psum accumulators live, gated behind the end of
            # attention so PSUM banks never overcommit.
            _pools8 = [ps_a, ps_a, ps_s, ps_s, ps_s, ps_acc, ps_acc, ps_acc]
            _tags8 = ["pa", "pa", "ps", "ps", "ps", "acc", "acc", "acc"]
            po8 = [
                _pools8[g].tile([128, 512], F32, tag=_tags8[g], name=f"pout{g}")
                for g in range(8)
            ]
            # phase A: kk-outer over first half of the contraction
            wo_tiles = {}
            for kk in range(8):
                w = wop.tile([128, D], F32R, tag="wo", name=f"wo{kk}")
                nc.sync.dma_start(w[:], wo_d[128 * kk : 128 * kk + 128, :])
                for g in range(8):
                    m, c = divmod(g, 2)
                    mm = nc.tensor.matmul(
                        po8[g][:],
                        w[:, 128 * m : 128 * m + 128],
                        oT[kk][:, 512 * c : 512 * c + 512],
                        start=(kk == 0),
                        stop=False,
                    )
                    if kk == 0:
                        add_dep_helper(
                            mm.ins,
                            last_attn.ins,
                            sync=False,
                            reason="out-proj psum after attention psum freed",
                        )
            # phase B: group-major so early groups finish, evict and DMA out
            # while later groups still accumulate
            for kk in range(8, 16):
                w = wop.tile([128, D], F32R, tag="wo", name=f"wo{kk}")
                nc.sync.dma_start(w[:], wo_d[128 * kk : 128 * kk + 128, :])
                wo_tiles[kk] = w
            for g in range(8):
                m, c = divmod(g, 2)
                for kk in range(8, 16):
                    nc.tensor.matmul(
                        po8[g][:],
                        wo_tiles[kk][:, 128 * m : 128 * m + 128],
                        oT[kk][:, 512 * c : 512 * c + 512],
                        start=False,
                        stop=(kk == 15),
                    )
                st = outst.tile([128, 512], F32, tag="outst")
                nc.scalar.activation(st[:], po8[g][:], ACT.Copy)
                nc.sync.dma_start(
                    outT_d[128 * m : 128 * m + 128, 512 * c : 512 * c + 512], st[:]
                )
            _ps_acc_cm.__exit__(None, None, None)
            _ps_s_cm.__exit__(None, None, None)
            attn_psum.__exit__(None, None, None)

    nc.compile()
    return nc


def _prep(Q, K, V, padding_mask, sequence_mask, Wq, bq, Wk, bk, Wv, bv, Wo, bo):
    assert padding_mask.min() == 1, "kernel assumes all-ones padding mask"
    seq = np.asarray(sequence_mask)
    tril = seq[0:128, 0:128].T.astype(np.float32)
    maskstrip = np.concatenate(
        [np.zeros((128, 128), np.float32), tril, np.ones((128, 384), np.float32)],
        axis=1,
    )
    for j in range(2):
        for i in range(4 * j, 4 * j + 4):
            r = i - 4 * j
            blk = seq[
                512 * j : 512 * j + 512, 128 * i : 128 * i + 128
            ].T.astype(np.float32)
            expect = np.concatenate(
                [
                    np.zeros((128, 128 * r), np.float32),
                    tril,
                    np.ones((128, 384 - 128 * r), np.float32),
                ],
                axis=1,
            )
            assert np.array_equal(blk, expect), "kernel assumes causal sequence mask"
        for i in range(4 * j):
            assert seq[512 * j : 512 * j + 512, 128 * i : 128 * i + 128].min() == 1
    c = np.ascontiguousarray
    shared = {
        "wq": c(Wq.astype(np.float32)),
        "wk": c(Wk.astype(np.float32)),
        "wv": c(Wv.astype(np.float32)),
        "wo": c(Wo.astype(np.float32)),
        "bq": c(bq.astype(np.float32)),
        "bk": c(bk.astype(np.float32)),
        "maskstrip": maskstrip,
        "onescol": np.ones((128, 1), np.float32),
    }
    in_maps = []
    for b in range(B):
        m = dict(shared)
        m["qT"] = c(np.asarray(Q[b]).T.astype(np.float32))
        m["kT"] = c(np.asarray(K[b]).T.astype(np.float32))
        m["vT"] = c(np.asarray(V[b]).T.astype(np.float32))
        in_maps.append(m)
    bo_eff = (
        np.asarray(bo, np.float32)
        + np.asarray(bv, np.float32).reshape(H * DV) @ np.asarray(Wo, np.float32)
    ).astype(np.float32)
    return in_maps, bo_eff


def kernel(Q, K, V, padding_mask, sequence_mask, Wq, bq, Wk, bk, Wv, bv, Wo, bo):
    if "nc" not in _CACHE:
        _CACHE["nc"] = build()
    nc = _CACHE["nc"]
    in_maps, bo_eff = _prep(
        Q, K, V, padding_mask, sequence_mask, Wq, bq, Wk, bk, Wv, bv, Wo, bo
    )
    res = run_bass_kernel_spmd(nc, in_maps, core_ids=list(range(B)))
    out = np.empty((B, S, D), np.float32)
    for b in range(B):
        out[b] = res.results[b]["outT"].T + bo_eff
    return out


# revision 62
# speedup vs baseline: 64.1452x; 1.0139x over previous
"""Trainium2 Bass kernel for 8-head causal MultiHeadAttention.

Problem (hardcoded): B=8, S=1024, d_model=512, H=8, d_k=128, d_v=256,
causal sequence mask, all-ones padding mask, fp32.

Strategy:
  - Batch-parallel across the 8 NeuronCores (1 batch element per core).
  - All matmuls in float32r (TF32-like fp32 @ 4x fp32 rate; ~13 mantissa
    bits) with every matmul free dim >= 256 for the full 1 cycle/row rate.
  - Scores are computed TRANSPOSED (S^T[t, q]) so the P@V contraction needs
    no transposes of the attention matrix; softmax denominators via a
    ones-vector matmul; normalization applied to O^T after the PV matmul;
    reciprocal broadcast across partitions with a rank-1 PE matmul.
  - Causality handled structurally (lower-triangular t-tiles only) plus
    0/1 mask multiplies on diagonal-band blocks.
  - Host side: transposes Q/K/V per batch element (so the kernel DMAs are
    contiguous), folds bv through softmax (rows sum to 1) and bo into a
    single host-side bias add, and transposes the per-core out^T back.
"""

import numpy as np

import concourse.bacc as bacc
import concourse.mybir as mybir
from concourse import tile
from concourse.bass_utils import run_bass_kernel_spmd
from concourse.tile_rust import add_dep_helper

B, S, D, H, DK, DV = 8, 1024, 512, 8, 128, 256
F32 = mybir.dt.float32
F32R = mybir.dt.float32r
ACT = mybir.ActivationFunctionType
SCALE = float(np.float32(1.0) / np.sqrt(np.float32(DK)).astype(np.float32))

_CACHE = {}


def build():
    nc = bacc.Bacc(trn_type="TRN2", target_bir_lowering=False, debug=False)

    qT_d = nc.dram_tensor("qT", [D, S], F32R, kind="ExternalInput").ap()
    kT_d = nc.dram_tensor("kT", [D, S], F32R, kind="ExternalInput").ap()
    vT_d = nc.dram_tensor("vT", [D, S], F32R, kind="ExternalInput").ap()
    wq_d = nc.dram_tensor("wq", [H, D, DK], F32R, kind="ExternalInput").ap()
    wk_d = nc.dram_tensor("wk", [H, D, DK], F32R, kind="ExternalInput").ap()
    wv_d = nc.dram_tensor("wv", [H, D, DV], F32R, kind="ExternalInput").ap()
    wo_d = nc.dram_tensor("wo", [H * DV, D], F32R, kind="ExternalInput").ap()
    bq_d = nc.dram_tensor("bq", [H, DK], F32, kind="ExternalInput").ap()
    bk_d = nc.dram_tensor("bk", [H, DK], F32, kind="ExternalInput").ap()
    mask_d = nc.dram_tensor("maskstrip", [128, 640], F32, kind="ExternalInput").ap()
    onescol_d = nc.dram_tensor("ones128", [128, 128], F32R, kind="ExternalInput").ap()
    outT_d = nc.dram_tensor("outT", [D, S], F32, kind="ExternalOutput").ap()

    with tile.TileContext(nc) as tc:
        with (
            tc.tile_pool(name="const", bufs=1) as const,
            tc.tile_pool(name="oTp", bufs=1) as oTp,
            tc.tile_pool(name="whead", bufs=2) as whead,
            tc.tile_pool(name="proj", bufs=2) as proj,
            tc.tile_pool(name="ptp", bufs=6) as ptp,
            tc.tile_pool(name="wop", bufs=8) as wop,
            tc.tile_pool(name="outst", bufs=2) as outst,
            tc.tile_pool(name="recipp", bufs=3) as recipp,
        ):
            attn_psum = tc.tile_pool(name="ps_a", bufs=2, space="PSUM")
            ps_a = attn_psum.__enter__()
            _ps_s_cm = tc.tile_pool(name="ps_s", bufs=3, space="PSUM")
            ps_s = _ps_s_cm.__enter__()
            _ps_acc_cm = tc.tile_pool(name="ps_acc", bufs=3, space="PSUM")
            ps_acc = _ps_acc_cm.__enter__()

            # ---- resident inputs ----
            def load_head_weights(h):
                bq_s = whead.tile([128, 1], F32, tag="bq", name=f"bq{h}")
                nc.sync.dma_start(bq_s[:], bq_d[h : h + 1, :].rearrange("o p -> p o"))
                bk_s = whead.tile([128, 1], F32, tag="bk", name=f"bk{h}")
                nc.sync.dma_start(bk_s[:], bk_d[h : h + 1, :].rearrange("o p -> p o"))
                wq_s = whead.tile([128, 4 * DK], F32R, tag="wq", name=f"wq{h}")
                nc.sync.dma_start(
                    wq_s[:].rearrange("p (k m) -> p k m", k=4),
                    wq_d[h].rearrange("(k p) m -> p k m", p=128),
                )
                wk_s = whead.tile([128, 4 * DK], F32R, tag="wk", name=f"wk{h}")
                nc.sync.dma_start(
                    wk_s[:].rearrange("p (k m) -> p k m", k=4),
                    wk_d[h].rearrange("(k p) m -> p k m", p=128),
                )
                wv_s = whead.tile([128, 4 * DV], F32R, tag="wv", name=f"wv{h}")
                nc.sync.dma_start(
                    wv_s[:].rearrange("p (k m) -> p k m", k=4),
                    wv_d[h].rearrange("(k p) m -> p k m", p=128),
                )
                return wq_s, wk_s, wv_s, bq_s, bk_s

            qTs, kTs, vTs = [], [], []
            for name, dram, lst in (("q", qT_d, qTs), ("k", kT_d, kTs), ("v", vT_d, vTs)):
                for k in range(4):
                    t = const.tile([128, S], F32R, tag=f"{name}T{k}", name=f"{name}T{k}")
                    lst.append(t)
            # startup-ordered loads: each projection's weight right before
            # the input tensor it contracts with
            # ACT-table warmup: a dummy activation at t~0 so LoadActFuncSet
            # doesn't serialize the first projection eviction
            warm = const.tile([128, 1], F32, tag="actwarm")
            nc.any.memset(warm[:], 0.0)
            nc.scalar.activation(warm[:], warm[:], ACT.Exp)
            wq_s0 = whead.tile([128, 4 * DK], F32R, tag="wq", name="wq0")
            nc.sync.dma_start(
                wq_s0[:].rearrange("p (k m) -> p k m", k=4),
                wq_d[0].rearrange("(k p) m -> p k m", p=128),
            )
            nc.sync.dma_start(qTs[0][:, 0:512], qT_d[0:128, 0:512])
            nc.sync.dma_start(qTs[0][:, 512:1024], qT_d[0:128, 512:1024])
            for k in range(1, 4):
                nc.sync.dma_start(qTs[k][:], qT_d[128 * k : 128 * k + 128, :])
            bq_s0 = whead.tile([128, 1], F32, tag="bq", name="bq0")
            nc.sync.dma_start(bq_s0[:], bq_d[0:1, :].rearrange("o p -> p o"))
            bk_s0 = whead.tile([128, 1], F32, tag="bk", name="bk0")
            nc.sync.dma_start(bk_s0[:], bk_d[0:1, :].rearrange("o p -> p o"))
            wk_s0 = whead.tile([128, 4 * DK], F32R, tag="wk", name="wk0")
            nc.sync.dma_start(
                wk_s0[:].rearrange("p (k m) -> p k m", k=4),
                wk_d[0].rearrange("(k p) m -> p k m", p=128),
            )
            nc.sync.dma_start(kTs[0][:, 0:512], kT_d[0:128, 0:512])
            nc.sync.dma_start(kTs[0][:, 512:1024], kT_d[0:128, 512:1024])
            for k in range(1, 4):
                nc.sync.dma_start(kTs[k][:], kT_d[128 * k : 128 * k + 128, :])
            head1_weights = load_head_weights(1)
            wv_s0 = whead.tile([128, 4 * DV], F32R, tag="wv", name="wv0")
            nc.sync.dma_start(
                wv_s0[:].rearrange("p (k m) -> p k m", k=4),
                wv_d[0].rearrange("(k p) m -> p k m", p=128),
            )
            for k in range(4):
                nc.sync.dma_start(vTs[k][:], vT_d[128 * k : 128 * k + 128, :])
            head0_weights = (wq_s0, wk_s0, wv_s0, bq_s0, bk_s0)
            mask_s = const.tile([128, 640], F32, tag="maskstrip")
            nc.sync.dma_start(mask_s[:], mask_d[:])
            onescol = const.tile([128, 128], F32R, tag="ones128")
            nc.sync.dma_start(onescol[:], onescol_d[:])

            oT = [oTp.tile([128, S], F32R, tag=f"oT{i}", name=f"oT{i}") for i in range(16)]

            # ---- per-head projections + attention (software-pipelined:
            # head h+1's Q/K projections are emitted before head h's V
            # projection so the PE never queues behind vT-gated work) ----
            def proj_qk(h, weights):
                wq_s, wk_s, _, bq_s, bk_s = weights
                qpT = proj.tile([128, S], F32R, tag="qpT", name=f"qpT{h}")
                kpT = proj.tile([128, S], F32R, tag="kpT", name=f"kpT{h}")
                for dst, w_s, src, b_s in ((qpT, wq_s, qTs, bq_s), (kpT, wk_s, kTs, bk_s)):
                    for c in range(2):
                        p = ps_a.tile([128, 512], F32, tag="pa")
                        for k in range(4):
                            nc.tensor.matmul(
                                p[:],
                                w_s[:, 128 * k : 128 * k + 128],
                                src[k][:, 512 * c : 512 * c + 512],
                                start=(k == 0),
                                stop=(k == 3),
                            )
                        if c == 0:
                            nc.scalar.activation(
                                dst[:, 512 * c : 512 * c + 512], p[:], ACT.Identity,
                                bias=b_s[:],
                            )
                        else:
                            nc.vector.tensor_scalar_add(
                                dst[:, 512 * c : 512 * c + 512], p[:], b_s[:]
                            )
                return qpT, kpT

            def proj_v(h, weights):
                wv_s = weights[2]
                vp = proj.tile([128, 8 * DV], F32R, tag="vp", name=f"vp{h}")
                for i in range(8):
                    p = ps_a.tile([128, DV], F32, tag="pa")
                    for k in range(4):
                        nc.tensor.matmul(
                            p[:],
                            vTs[k][:, 128 * i : 128 * i + 128],
                            wv_s[:, DV * k : DV * k + DV],
                            start=(k == 0),
                            stop=(k == 3),
                        )
                    if i % 2 == 0:
                        nc.scalar.activation(
                            vp[:, DV * i : DV * i + DV], p[:], ACT.Copy
                        )
                    else:
                        nc.vector.tensor_copy(vp[:, DV * i : DV * i + DV], p[:])
                return vp

            def attn(h, qpT, kpT, vp):
                last = None
                # attention per 512-wide q-chunk
                for j in range(2):
                    n_t = 4 * (j + 1)
                    qlo = 512 * j
                    po = [
                        ps_acc.tile([128, 512], F32, tag="acc", name=f"po{vh}")
                        for vh in range(2)
                    ]
                    pr = ps_acc.tile([128, 512], F32, tag="acc", name="pr")
                    for i in range(n_t):
                        # live column window: causality kills q < 128*r in
                        # this t-tile; round the window down to >=256 wide so
                        # f32r stays at full rate
                        r = i - 4 * j
                        wlo = 0 if r < 1 else min(128 * r, 256)
                        nw = 512 - wlo
                        psc = ps_s.tile([128, nw], F32, tag="ps", name=f"psc{i}")
                        nc.tensor.matmul(
                            psc[:],
                            kpT[:, 128 * i : 128 * i + 128],
                            qpT[:, qlo + wlo : qlo + 512],
                            start=True,
                            stop=True,
                        )
                        pt = ptp.tile([128, nw], F32R, tag="pt", name=f"pt{i}")
                        nc.scalar.activation(pt[:], psc[:], ACT.Exp, scale=SCALE)
                        if 0 <= r <= 2:
                            lo = 128 * r - wlo
                            nc.vector.tensor_mul(
                                pt[:, lo : lo + 128],
                                pt[:, lo : lo + 128],
                                mask_s[:, 128:256],
                            )
                        elif r == 3:
                            nc.vector.tensor_mul(
                                pt[:, 0:256],
                                pt[:, 0:256],
                                mask_s[:, 0:256],
                            )
                        for vh in range(2):
                            nc.tensor.matmul(
                                po[vh][:, wlo:512],
                                vp[:, DV * i + 128 * vh : DV * i + 128 * vh + 128],
                                pt[:],
                                start=(i == 0),
                                stop=(i == n_t - 1),
                                skip_group_check=True,
                            )
                        nc.tensor.matmul(
                            pr[:, wlo:512],
                            onescol[:],
                            pt[:],
                            start=(i == 0),
                            stop=(i == n_t - 1),
                            skip_group_check=True,
                        )
                    pbs = recipp.tile([128, 512], F32, tag="pbs")
                    nc.vector.reciprocal(pbs[:], pr[:])
                    for vh in range(2):
                        mm = nc.vector.tensor_mul(
                            oT[2 * h + vh][:, qlo : qlo + 512], po[vh][:], pbs[:]
                        )
                        last = mm
                return last

            weights = {0: head0_weights, 1: head1_weights}
            for h in range(H):
                if h not in weights:
                    weights[h] = load_head_weights(h)
                qpT_h, kpT_h = proj_qk(h, weights[h])
                vp_h = proj_v(h, weights[h])
                last_attn = attn(h, qpT_h, kpT_h, vp_h)

            # ---- output projection: outT[m, s] ----
            # kk outer so each wo tile is consumed in one burst (4 wop slots
            # suffice); 8 psum accumulators live, gated behind the end of
            # attention so PSUM banks never overcommit.
            _pools8 = [ps_a, ps_a, ps_s, ps_s, ps_s, ps_acc, ps_acc, ps_acc]
            _tags8 = ["pa", "pa", "ps", "ps", "ps", "acc", "acc", "acc"]
            po8 = [
                _pools8[g].tile([128, 512], F32, tag=_tags8[g], name=f"pout{g}")
                for g in range(8)
            ]
            # phase A: kk-outer over first half of the contraction
            wo_tiles = {}
            for kk in range(8):
                w = wop.tile([128, D], F32R, tag="wo", name=f"wo{kk}")
                nc.sync.dma_start(w[:], wo_d[128 * kk : 128 * kk + 128, :])
                for g in range(8):
                    m, c = divmod(g, 2)
                    mm = nc.tensor.matmul(
                        po8[g][:],
                        w[:, 128 * m : 128 * m + 128],
                        oT[kk][:, 512 * c : 512 * c + 512],
                        start=(kk == 0),
                        stop=False,
                    )
                    if kk == 0:
                        add_dep_helper(
                            mm.ins,
                            last_attn.ins,
                            sync=False,
                            reason="out-proj psum after attention psum freed",
                        )
            # phase B: group-major so early groups finish, evict and DMA out
            # while later groups still accumulate
            for kk in range(8, 16):
                w = wop.tile([128, D], F32R, tag="wo", name=f"wo{kk}")
                nc.sync.dma_start(w[:], wo_d[128 * kk : 128 * kk + 128, :])
                wo_tiles[kk] = w
            for g in range(8):
                m, c = divmod(g, 2)
                for kk in range(8, 16):
                    nc.tensor.matmul(
                        po8[g][:],
                        wo_tiles[kk][:, 128 * m : 128 * m + 128],
                        oT[kk][:, 512 * c : 512 * c + 512],
                        start=False,
                        stop=(kk == 15),
                    )
                st = outst.tile([128, 512], F32, tag="outst")
                nc.scalar.activation(st[:], po8[g][:], ACT.Copy)
                nc.sync.dma_start(
                    outT_d[128 * m : 128 * m + 128, 512 * c : 512 * c + 512], st[:]
                )
            _ps_acc_cm.__exit__(None, None, None)
            _ps_s_cm.__exit__(None, None, None)
            attn_psum.__exit__(None, None, None)

    nc.compile()
    return nc


def _prep(Q, K, V, padding_mask, sequence_mask, Wq, bq, Wk, bk, Wv, bv, Wo, bo):
    assert padding_mask.min() == 1, "kernel assumes all-ones padding mask"
    seq = np.asarray(sequence_mask)
    tril = seq[0:128, 0:128].T.astype(np.float32)
    maskstrip = np.concatenate(
        [np.zeros((128, 128), np.float32), tril, np.ones((128, 384), np.float32)],
        axis=1,
    )
    for j in range(2):
        for i in range(4 * j, 4 * j + 4):
            r = i - 4 * j
            blk = seq[
                512 * j : 512 * j + 512, 128 * i : 128 * i + 128
            ].T.astype(np.float32)
            expect = np.concatenate(
                [
                    np.zeros((128, 128 * r), np.float32),
                    tril,
                    np.ones((128, 384 - 128 * r), np.float32),
                ],
                axis=1,
            )
            assert np.array_equal(blk, expect), "kernel assumes causal sequence mask"
        for i in range(4 * j):
            assert seq[512 * j : 512 * j + 512, 128 * i : 128 * i + 128].min() == 1
    c = np.ascontiguousarray
    shared = {
        "wq": c(Wq.astype(np.float32)),
        "wk": c(Wk.astype(np.float32)),
        "wv": c(Wv.astype(np.float32)),
        "wo": c(Wo.astype(np.float32)),
        "bq": c(bq.astype(np.float32)),
        "bk": c(bk.astype(np.float32)),
        "maskstrip": maskstrip,
        "ones128": np.ones((128, 128), np.float32),
    }
    in_maps = []
    for b in range(B):
        m = dict(shared)
        m["qT"] = c(np.asarray(Q[b]).T.astype(np.float32))
        m["kT"] = c(np.asarray(K[b]).T.astype(np.float32))
        m["vT"] = c(np.asarray(V[b]).T.astype(np.float32))
        in_maps.append(m)
    bo_eff = (
        np.asarray(bo, np.float32)
        + np.asarray(bv, np.float32).reshape(H * DV) @ np.asarray(Wo, np.float32)
    ).astype(np.float32)
    return in_maps, bo_eff


def kernel(Q, K, V, padding_mask, sequence_mask, Wq, bq, Wk, bk, Wv, bv, Wo, bo):
    if "nc" not in _CACHE:
        _CACHE["nc"] = build()
    nc = _CACHE["nc"]
    in_maps, bo_eff = _prep(
        Q, K, V, padding_mask, sequence_mask, Wq, bq, Wk, bk, Wv, bv, Wo, bo
    )
    res = run_bass_kernel_spmd(nc, in_maps, core_ids=list(range(B)))
    out = np.empty((B, S, D), np.float32)
    for b in range(B):
        out[b] = res.results[b]["outT"].T + bo_eff
    return out


# revision 68
# speedup vs baseline: 64.4711x; 1.0051x over previous
"""Trainium2 Bass kernel for 8-head causal MultiHeadAttention.

Problem (hardcoded): B=8, S=1024, d_model=512, H=8, d_k=128, d_v=256,
causal sequence mask, all-ones padding mask, fp32.

Strategy:
  - Batch-parallel across the 8 NeuronCores (1 batch element per core).
  - All matmuls in float32r (TF32-like fp32 @ 4x fp32 rate; ~13 mantissa
    bits) with every matmul free dim >= 256 for the full 1 cycle/row rate.
  - Scores are computed TRANSPOSED (S^T[t, q]) so the P@V contraction needs
    no transposes of the attention matrix; softmax denominators via a
    ones-vector matmul; normalization applied to O^T after the PV matmul;
    reciprocal broadcast across partitions with a rank-1 PE matmul.
  - Causality handled structurally (lower-triangular t-tiles only) plus
    0/1 mask multiplies on diagonal-band blocks.
  - Host side: transposes Q/K/V per batch element (so the kernel DMAs are
    contiguous), folds bv through softmax (rows sum to 1) and bo into a
    single host-side bias add, and transposes the per-core out^T back.
"""

import numpy as np

import concourse.bacc as bacc
import concourse.mybir as mybir
from concourse import tile
from concourse.bass_utils import run_bass_kernel_spmd
from concourse.tile_rust import add_dep_helper

B, S, D, H, DK, DV = 8, 1024, 512, 8, 128, 256
F32 = mybir.dt.float32
F32R = mybir.dt.float32r
ACT = mybir.ActivationFunctionType
SCALE = float(np.float32(1.0) / np.sqrt(np.float32(DK)).astype(np.float32))

_CACHE = {}


def build():
    nc = bacc.Bacc(trn_type="TRN2", target_bir_lowering=False, debug=False)

    qT_d = nc.dram_tensor("qT", [D, S], F32R, kind="ExternalInput").ap()
    kT_d = nc.dram_tensor("kT", [D, S], F32R, kind="ExternalInput").ap()
    vT_d = nc.dram_tensor("vT", [D, S], F32R, kind="ExternalInput").ap()
    wq_d = nc.dram_tensor("wq", [H, D, DK], F32R, kind="ExternalInput").ap()
    wk_d = nc.dram_tensor("wk", [H, D, DK], F32R, kind="ExternalInput").ap()
    wv_d = nc.dram_tensor("wv", [H, D, DV], F32R, kind="ExternalInput").ap()
    wo_d = nc.dram_tensor("wo", [H * DV, D], F32R, kind="ExternalInput").ap()
    bq_d = nc.dram_tensor("bq", [H, DK], F32, kind="ExternalInput").ap()
    bk_d = nc.dram_tensor("bk", [H, DK], F32, kind="ExternalInput").ap()
    mask_d = nc.dram_tensor("maskstrip", [128, 640], F32, kind="ExternalInput").ap()
    onescol_d = nc.dram_tensor("ones128", [128, 128], F32R, kind="ExternalInput").ap()
    outT_d = nc.dram_tensor("outT", [D, S], F32, kind="ExternalOutput").ap()

    with tile.TileContext(nc) as tc:
        with (
            tc.tile_pool(name="const", bufs=1) as const,
            tc.tile_pool(name="oTp", bufs=1) as oTp,
            tc.tile_pool(name="whead", bufs=2) as whead,
            tc.tile_pool(name="proj", bufs=2) as proj,
            tc.tile_pool(name="ptp", bufs=9) as ptp,
            tc.tile_pool(name="wop", bufs=8) as wop,
            tc.tile_pool(name="outst", bufs=2) as outst,
            tc.tile_pool(name="recipp", bufs=2) as recipp,
        ):
            attn_psum = tc.tile_pool(name="ps_a", bufs=2, space="PSUM")
            ps_a = attn_psum.__enter__()
            _ps_s_cm = tc.tile_pool(name="ps_s", bufs=3, space="PSUM")
            ps_s = _ps_s_cm.__enter__()
            _ps_acc_cm = tc.tile_pool(name="ps_acc", bufs=3, space="PSUM")
            ps_acc = _ps_acc_cm.__enter__()

            # ---- resident inputs ----
            def load_head_weights(h):
                bq_s = whead.tile([128, 1], F32, tag="bq", name=f"bq{h}")
                nc.sync.dma_start(bq_s[:], bq_d[h : h + 1, :].rearrange("o p -> p o"))
                bk_s = whead.tile([128, 1], F32, tag="bk", name=f"bk{h}")
                nc.sync.dma_start(bk_s[:], bk_d[h : h + 1, :].rearrange("o p -> p o"))
                wq_s = whead.tile([128, 4 * DK], F32R, tag="wq", name=f"wq{h}")
                nc.sync.dma_start(
                    wq_s[:].rearrange("p (k m) -> p k m", k=4),
                    wq_d[h].rearrange("(k p) m -> p k m", p=128),
                )
                wk_s = whead.tile([128, 4 * DK], F32R, tag="wk", name=f"wk{h}")
                nc.sync.dma_start(
                    wk_s[:].rearrange("p (k m) -> p k m", k=4),
                    wk_d[h].rearrange("(k p) m -> p k m", p=128),
                )
                wv_s = whead.tile([128, 4 * DV], F32R, tag="wv", name=f"wv{h}")
                nc.sync.dma_start(
                    wv_s[:].rearrange("p (k m) -> p k m", k=4),
                    wv_d[h].rearrange("(k p) m -> p k m", p=128),
                )
                return wq_s, wk_s, wv_s, bq_s, bk_s

            qTs, kTs, vTs = [], [], []
            for name, dram, lst in (("q", qT_d, qTs), ("k", kT_d, kTs), ("v", vT_d, vTs)):
                for k in range(4):
                    t = const.tile([128, S], F32R, tag=f"{name}T{k}", name=f"{name}T{k}")
                    lst.append(t)
            # startup-ordered loads: each projection's weight right before
            # the input tensor it contracts with
            # ACT-table warmup: a dummy activation at t~0 so LoadActFuncSet
            # doesn't serialize the first projection eviction
            warm = const.tile([128, 1], F32, tag="actwarm")
            nc.any.memset(warm[:], 0.0)
            nc.scalar.activation(warm[:], warm[:], ACT.Exp)
            wq_s0 = whead.tile([128, 4 * DK], F32R, tag="wq", name="wq0")
            nc.sync.dma_start(
                wq_s0[:].rearrange("p (k m) -> p k m", k=4),
                wq_d[0].rearrange("(k p) m -> p k m", p=128),
            )
            nc.sync.dma_start(qTs[0][:, 0:512], qT_d[0:128, 0:512])
            nc.sync.dma_start(qTs[0][:, 512:1024], qT_d[0:128, 512:1024])
            for k in range(1, 4):
                nc.sync.dma_start(qTs[k][:], qT_d[128 * k : 128 * k + 128, :])
            bq_s0 = whead.tile([128, 1], F32, tag="bq", name="bq0")
            nc.sync.dma_start(bq_s0[:], bq_d[0:1, :].rearrange("o p -> p o"))
            bk_s0 = whead.tile([128, 1], F32, tag="bk", name="bk0")
            nc.sync.dma_start(bk_s0[:], bk_d[0:1, :].rearrange("o p -> p o"))
            wk_s0 = whead.tile([128, 4 * DK], F32R, tag="wk", name="wk0")
            nc.sync.dma_start(
                wk_s0[:].rearrange("p (k m) -> p k m", k=4),
                wk_d[0].rearrange("(k p) m -> p k m", p=128),
            )
            nc.sync.dma_start(kTs[0][:, 0:512], kT_d[0:128, 0:512])
            nc.sync.dma_start(kTs[0][:, 512:1024], kT_d[0:128, 512:1024])
            for k in range(1, 4):
                nc.sync.dma_start(kTs[k][:], kT_d[128 * k : 128 * k + 128, :])
            head1_weights = load_head_weights(1)
            wv_s0 = whead.tile([128, 4 * DV], F32R, tag="wv", name="wv0")
            nc.sync.dma_start(
                wv_s0[:].rearrange("p (k m) -> p k m", k=4),
                wv_d[0].rearrange("(k p) m -> p k m", p=128),
            )
            for k in range(4):
                nc.sync.dma_start(vTs[k][:], vT_d[128 * k : 128 * k + 128, :])
            head0_weights = (wq_s0, wk_s0, wv_s0, bq_s0, bk_s0)
            mask_s = const.tile([128, 640], F32, tag="maskstrip")
            nc.sync.dma_start(mask_s[:], mask_d[:])
            onescol = const.tile([128, 128], F32R, tag="ones128")
            nc.sync.dma_start(onescol[:], onescol_d[:])

            oT = [oTp.tile([128, S], F32R, tag=f"oT{i}", name=f"oT{i}") for i in range(16)]

            # ---- per-head projections + attention (software-pipelined:
            # head h+1's Q/K projections are emitted before head h's V
            # projection so the PE never queues behind vT-gated work) ----
            def proj_qk(h, weights):
                wq_s, wk_s, _, bq_s, bk_s = weights
                qpT = proj.tile([128, S], F32R, tag="qpT", name=f"qpT{h}")
                kpT = proj.tile([128, S], F32R, tag="kpT", name=f"kpT{h}")
                for dst, w_s, src, b_s in ((qpT, wq_s, qTs, bq_s), (kpT, wk_s, kTs, bk_s)):
                    for c in range(2):
                        p = ps_a.tile([128, 512], F32, tag="pa")
                        for k in range(4):
                            nc.tensor.matmul(
                                p[:],
                                w_s[:, 128 * k : 128 * k + 128],
                                src[k][:, 512 * c : 512 * c + 512],
                                start=(k == 0),
                                stop=(k == 3),
                            )
                        if c == 0:
                            nc.scalar.activation(
                                dst[:, 512 * c : 512 * c + 512], p[:], ACT.Identity,
                                bias=b_s[:],
                            )
                        else:
                            nc.vector.tensor_scalar_add(
                                dst[:, 512 * c : 512 * c + 512], p[:], b_s[:]
                            )
                return qpT, kpT

            def proj_v(h, weights):
                wv_s = weights[2]
                vp = proj.tile([128, 8 * DV], F32R, tag="vp", name=f"vp{h}")
                for i in range(8):
                    p = ps_a.tile([128, DV], F32, tag="pa")
                    for k in range(4):
                        nc.tensor.matmul(
                            p[:],
                            vTs[k][:, 128 * i : 128 * i + 128],
                            wv_s[:, DV * k : DV * k + DV],
                            start=(k == 0),
                            stop=(k == 3),
                        )
                    if i % 2 == 0:
                        nc.scalar.activation(
                            vp[:, DV * i : DV * i + DV], p[:], ACT.Copy
                        )
                    else:
                        nc.vector.tensor_copy(vp[:, DV * i : DV * i + DV], p[:])
                return vp

            def attn(h, qpT, kpT, vp):
                last = None
                # attention per 512-wide q-chunk
                for j in range(2):
                    n_t = 4 * (j + 1)
                    qlo = 512 * j
                    po = [
                        ps_acc.tile([128, 512], F32, tag="acc", name=f"po{vh}")
                        for vh in range(2)
                    ]
                    pr = ps_acc.tile([128, 512], F32, tag="acc", name="pr")
                    for i in range(n_t):
                        # live column window: causality kills q < 128*r in
                        # this t-tile; round the window down to >=256 wide so
                        # f32r stays at full rate
                        r = i - 4 * j
                        wlo = 0 if r < 1 else min(128 * r, 256)
                        nw = 512 - wlo
                        psc = ps_s.tile([128, nw], F32, tag="ps", name=f"psc{i}")
                        nc.tensor.matmul(
                            psc[:],
                            kpT[:, 128 * i : 128 * i + 128],
                            qpT[:, qlo + wlo : qlo + 512],
                            start=True,
                            stop=True,
                        )
                        pt = ptp.tile([128, nw], F32R, tag="pt", name=f"pt{i}")
                        nc.scalar.activation(pt[:], psc[:], ACT.Exp, scale=SCALE)
                        if 0 <= r <= 2:
                            lo = 128 * r - wlo
                            nc.vector.tensor_mul(
                                pt[:, lo : lo + 128],
                                pt[:, lo : lo + 128],
                                mask_s[:, 128:256],
                            )
                        elif r == 3:
                            nc.vector.tensor_mul(
                                pt[:, 0:256],
                                pt[:, 0:256],
                                mask_s[:, 0:256],
                            )
                        for vh in range(2):
                            nc.tensor.matmul(
                                po[vh][:, wlo:512],
                                vp[:, DV * i + 128 * vh : DV * i + 128 * vh + 128],
                                pt[:],
                                start=(i == 0),
                                stop=(i == n_t - 1),
                                skip_group_check=True,
                            )
                        nc.tensor.matmul(
                            pr[:, wlo:512],
                            onescol[:],
                            pt[:],
                            start=(i == 0),
                            stop=(i == n_t - 1),
                            skip_group_check=True,
                        )
                    pbs = recipp.tile([128, 512], F32, tag="pbs")
                    nc.vector.reciprocal(pbs[:], pr[:])
                    for vh in range(2):
                        mm = nc.vector.tensor_mul(
                            oT[2 * h + vh][:, qlo : qlo + 512], po[vh][:], pbs[:]
                        )
                        last = mm
                return last

            weights = {0: head0_weights, 1: head1_weights}
            for h in range(H):
                if h not in weights:
                    weights[h] = load_head_weights(h)
                qpT_h, kpT_h = proj_qk(h, weights[h])
                vp_h = proj_v(h, weights[h])
                last_attn = attn(h, qpT_h, kpT_h, vp_h)

            # ---- output projection: outT[m, s] ----
            # kk outer so each wo tile is consumed in one burst (4 wop slots
            # suffice); 8 psum accumulators live, gated behind the end of
            # attention so PSUM banks never overcommit.
            _pools8 = [ps_a, ps_a, ps_s, ps_s, ps_s, ps_acc, ps_acc, ps_acc]
            _tags8 = ["pa", "pa", "ps", "ps", "ps", "acc", "acc", "acc"]
            po8 = [
                _pools8[g].tile([128, 512], F32, tag=_tags8[g], name=f"pout{g}")
                for g in range(8)
            ]
            # phase A: kk-outer over first half of the contraction
            wo_tiles = {}
            for kk in range(8):
                w = wop.tile([128, D], F32R, tag="wo", name=f"wo{kk}")
                nc.sync.dma_start(w[:], wo_d[128 * kk : 128 * kk + 128, :])
                for g in range(8):
                    m, c = divmod(g, 2)
                    mm = nc.tensor.matmul(
                        po8[g][:],
                        w[:, 128 * m : 128 * m + 128],
                        oT[kk][:, 512 * c : 512 * c + 512],
                        start=(kk == 0),
                        stop=False,
                    )
                    if kk == 0:
                        add_dep_helper(
                            mm.ins,
                            last_attn.ins,
                            sync=False,
                            reason="out-proj psum after attention psum freed",
                        )
            # phase B: group-major so early groups finish, evict and DMA out
            # while later groups still accumulate
            for kk in range(8, 16):
                w = wop.tile([128, D], F32R, tag="wo", name=f"wo{kk}")
                nc.sync.dma_start(w[:], wo_d[128 * kk : 128 * kk + 128, :])
                wo_tiles[kk] = w
            for g in range(8):
                m, c = divmod(g, 2)
                for kk in range(8, 16):
                    nc.tensor.matmul(
                        po8[g][:],
                        wo_tiles[kk][:, 128 * m : 128 * m + 128],
                        oT[kk][:, 512 * c : 512 * c + 512],
                        start=False,
                        stop=(kk == 15),
                    )
                st = outst.tile([128, 512], F32, tag="outst")
                nc.scalar.activation(st[:], po8[g][:], ACT.Copy)
                nc.sync.dma_start(
                    outT_d[128 * m : 128 * m + 128, 512 * c : 512 * c + 512], st[:]
                )
            _ps_acc_cm.__exit__(None, None, None)
            _ps_s_cm.__exit__(None, None, None)
            attn_psum.__exit__(None, None, None)

    nc.compile()
    return nc


def _prep(Q, K, V, padding_mask, sequence_mask, Wq, bq, Wk, bk, Wv, bv, Wo, bo):
    assert padding_mask.min() == 1, "kernel assumes all-ones padding mask"
    seq = np.asarray(sequence_mask)
    tril = seq[0:128, 0:128].T.astype(np.float32)
    maskstrip = np.concatenate(
        [np.zeros((128, 128), np.float32), tril, np.ones((128, 384), np.float32)],
        axis=1,
    )
    for j in range(2):
        for i in range(4 * j, 4 * j + 4):
            r = i - 4 * j
            blk = seq[
                512 * j : 512 * j + 512, 128 * i : 128 * i + 128
            ].T.astype(np.float32)
            expect = np.concatenate(
                [
                    np.zeros((128, 128 * r), np.float32),
                    tril,
                    np.ones((128, 384 - 128 * r), np.float32),
                ],
                axis=1,
            )
            assert np.array_equal(blk, expect), "kernel assumes causal sequence mask"
        for i in range(4 * j):
            assert seq[512 * j : 512 * j + 512, 128 * i : 128 * i + 128].min() == 1
    c = np.ascontiguousarray
    shared = {
        "wq": c(Wq.astype(np.float32)),
        "wk": c(Wk.astype(np.float32)),
        "wv": c(Wv.astype(np.float32)),
        "wo": c(Wo.astype(np.float32)),
        "bq": c(bq.astype(np.float32)),
        "bk": c(bk.astype(np.float32)),
        "maskstrip": maskstrip,
        "ones128": np.ones((128, 128), np.float32),
    }
    in_maps = []
    for b in range(B):
        m = dict(shared)
        m["qT"] = c(np.asarray(Q[b]).T.astype(np.float32))
        m["kT"] = c(np.asarray(K[b]).T.astype(np.float32))
        m["vT"] = c(np.asarray(V[b]).T.astype(np.float32))
        in_maps.append(m)
    bo_eff = (
        np.asarray(bo, np.float32)
        + np.asarray(bv, np.float32).reshape(H * DV) @ np.asarray(Wo, np.float32)
    ).astype(np.float32)
    return in_maps, bo_eff


def kernel(Q, K, V, padding_mask, sequence_mask, Wq, bq, Wk, bk, Wv, bv, Wo, bo):
    if "nc" not in _CACHE:
        _CACHE["nc"] = build()
    nc = _CACHE["nc"]
    in_maps, bo_eff = _prep(
        Q, K, V, padding_mask, sequence_mask, Wq, bq, Wk, bk, Wv, bv, Wo, bo
    )
    res = run_bass_kernel_spmd(nc, in_maps, core_ids=list(range(B)))
    out = np.empty((B, S, D), np.float32)
    for b in range(B):
        out[b] = res.results[b]["outT"].T + bo_eff
    return out
